# revision 17
# baseline (speedup 1.0000x reference)
"""CodeGen attention block (B=1, S=2048, E=2048, H=16, D=128, rot=64) on 8 TRN2
NeuronCores.

Sharding: tensor-parallel over heads (2 heads/core). Each core computes its
heads' q/k in transposed [d, s] layout (partial rotary applied via a host-side
even/odd channel permutation folded into the qkv weights, which turns the
interleaved rotation into a rotate-half form on contiguous partition blocks),
v in [s, d] layout, causal softmax attention entirely on-chip (scores
transposed [k, q] so the P·V matmul needs no transposes; softmax denominators
via a ones-stationary matmul), then AllGathers the per-core attention output
O^T [256, 2048] and computes a 256-column slice of the output projection.
Host assembles the 8 slices and transposes.

All PE-facing tensors are bf16 (full PE speed, half the DMA); accumulation
is fp32 in PSUM. The AllGather is split per-s-chunk (and per-head for the
last chunk) so collectives fire as soon as the producing attention work
finishes, overlapping comm with attention; the output projection for chunk j
is interleaved into the main loop after chunk j+2's attention so PE never
waits on a collective except at the very tail.
"""

import numpy as np

H, D, ROT, MP = 16, 128, 64, 4
S, E = 2048, 2048
NCORES = 8
P = 128
NQ = 4            # 512-wide q chunks
NKT = S // P      # 16 k tiles
NEC = E // P      # 16 contraction chunks
SCALE = float(1.0 / np.sqrt(np.float64(D)))

_CACHE = {}


# ----------------------------------------------------------------------------
# host-side input prep
# ----------------------------------------------------------------------------

def _head_rows(h):
    g, j = h // 4, h % 4
    base = g * (3 * 512)
    q = np.arange(base + j * 128, base + (j + 1) * 128)
    v = np.arange(base + 512 + j * 128, base + 512 + (j + 1) * 128)
    k = np.arange(base + 1024 + j * 128, base + 1024 + (j + 1) * 128)
    return q, v, k


def _prep_core_weights(c, w_qkv, w_out):
    h0, h1 = 2 * c, 2 * c + 1
    top = np.arange(0, ROT, 2)
    bot = np.arange(1, ROT, 2)
    wq, wk, wv = {}, {}, {}
    for h in (h0, h1):
        qr, vr, kr = _head_rows(h)
        wq[h], wv[h], wk[h] = w_qkv[qr], w_qkv[vr], w_qkv[kr]
    G0 = np.concatenate([wq[h0][top], wq[h1][top], wk[h0][top], wk[h1][top]], 0)
    G1 = np.concatenate([wq[h0][bot], wq[h1][bot], wk[h0][bot], wk[h1][bot]], 0)
    G2 = np.concatenate([wq[h0][ROT:], wq[h1][ROT:]], 0)
    G3 = np.concatenate([wk[h0][ROT:], wk[h1][ROT:]], 0)
    wqkT = np.ascontiguousarray(
        np.concatenate([G0, G1, G2, G3], 0).T, dtype=np.float32)       # [E, 512]
    wvT = np.ascontiguousarray(
        np.concatenate([wv[h0], wv[h1]], 0).T, dtype=np.float32)       # [E, 256]
    woutT = np.ascontiguousarray(
        w_out[256 * c:256 * (c + 1), :].T, dtype=np.float32)           # [E, 256]
    # pre-pack for a fully contiguous [128, 16, 256] SBUF load
    woutT = np.ascontiguousarray(
        woutT.reshape(16, 128, 256).transpose(1, 0, 2)).reshape(128, 16 * 256)
    return wqkT, wvT, woutT


def _cos_sin():
    inv_freq = 1.0 / (10000.0 ** (np.arange(0, ROT, 2, dtype=np.float32) / ROT))
    ang = np.arange(S, dtype=np.float32)[:, None] * inv_freq[None, :]
    cosb = np.cos(ang).T.astype(np.float32)        # [32, S]
    sinb = np.sin(ang).T.astype(np.float32)
    return (np.ascontiguousarray(np.tile(cosb, (4, 1))),
            np.ascontiguousarray(np.tile(sinb, (4, 1))))               # [128, S]


def _mask_tiles():
    # single shifted mask: M[p, x] = (p <= x - 384); the diagonal-offset-mm
    # mask is the 512-wide view starting at column 384 - 128*mm
    pp = np.arange(128)[:, None]
    xx = np.arange(896)[None, :]
    return (xx >= pp + 384).astype(np.float32)


def _build_in_maps(hidden_states, w_qkv, w_out):
    import ml_dtypes
    bf16 = ml_dtypes.bfloat16
    hiddenT = np.ascontiguousarray(
        np.asarray(hidden_states, np.float32).reshape(S, E).T).astype(bf16)
    COS, SIN = _cos_sin()
    COS, SIN = COS.astype(bf16), SIN.astype(bf16)
    masks = _mask_tiles().astype(bf16)
    in_maps = []
    for c in range(NCORES):
        wqkT, wvT, woutT = _prep_core_weights(
            c, np.asarray(w_qkv, np.float32), np.asarray(w_out, np.float32))
        in_maps.append({
            "hiddenT": hiddenT,
            "wqkT": wqkT.astype(bf16),
            "wvT": wvT.astype(bf16),
            "woutT": woutT.astype(bf16),
            "cosT": COS,
            "sinT": SIN,
            "masks": masks,
        })
    return in_maps


# ----------------------------------------------------------------------------
# device program
# ----------------------------------------------------------------------------

def _kernel_body(tc, outT, hiddenT, wqkT, wvT, woutT, cosT, sinT, masksT):
    import concourse.mybir as mybir
    from contextlib import ExitStack

    nc = tc.nc
    f32 = mybir.dt.float32
    bt = mybir.dt.bfloat16

    with ExitStack() as ctx:
        const = ctx.enter_context(tc.tile_pool(name="const", bufs=1))
        mask_sb = const.tile([P, 896], bt, name="mask_sb")
        ones_sb = const.tile([P, P], bt, name="ones_sb")
        wo_sb = const.tile([P, NEC, 256], bt, name="wo_sb")
        nc.vector.memset(ones_sb[:], 1.0)

        dram = ctx.enter_context(tc.tile_pool(name="dram", bufs=1, space="DRAM"))
        ag_in = [dram.tile([2 * P, 512], bt, name=f"ag_in{j}", tag=f"ag_in{j}")
                 for j in range(NQ - 1)]
        ag_out = [dram.tile([E, 512], bt, name=f"ag_out{j}", tag=f"ag_out{j}",
                            addr_space="Shared") for j in range(NQ - 1)]
        # last chunk: per-head collectives so comm starts as soon as head 0
        # finishes and the tail only waits on head 1's small AG
        ag_in3 = [dram.tile([P, 512], bt, name=f"ag_in3{h}", tag=f"ag_in3{h}")
                  for h in range(2)]
        ag_out3 = [dram.tile([8 * P, 512], bt, name=f"ag_out3{h}",
                             tag=f"ag_out3{h}", addr_space="Shared")
                   for h in range(2)]

        # --- phase 1+2 interleaved: per-chunk qkv -> attention -> AG --------
        with tc.tile_pool(name="ph1c", bufs=1) as ph1c_pool, \
             tc.tile_pool(name="chunk", bufs=1) as ck_pool, \
             tc.tile_pool(name="hid", bufs=1) as hid_pool, \
             tc.tile_pool(name="wts", bufs=1) as wts_pool, \
             tc.tile_pool(name="g01c", bufs=1) as g01_pool, \
             tc.tile_pool(name="rtmp", bufs=1) as rtmp_pool, \
             tc.tile_pool(name="pt", bufs=6) as pt_pool, \
             tc.tile_pool(name="oout", bufs=2) as oo_pool, \
             tc.tile_pool(name="otin", bufs=2) as oi_pool, \
             tc.tile_pool(name="otin3", bufs=1) as oi3_pool, \
             tc.tile_pool(name="oprj", bufs=2) as op_pool, \
             tc.tile_pool(name="gps", bufs=2, space="PSUM") as gps_pool, \
             tc.tile_pool(name="scps", bufs=2, space="PSUM") as sc_pool, \
             tc.tile_pool(name="otps", bufs=2, space="PSUM") as ot_pool, \
             tc.tile_pool(name="dnps", bufs=2, space="PSUM") as dn_pool:

            pp_pool = gps_pool  # oproj PSUM reuses the (idle-by-then) qkv banks

            cos_sb = ph1c_pool.tile([P, S], bt, name="cos_sb", tag="cos_sb")
            sin_sb = ph1c_pool.tile([P, S], bt, name="sin_sb", tag="sin_sb")

            # per-chunk activations: q/k transposed [d, 512]; v [k-in-tile, 4*128]
            # q is only live from its qkv until its attention -> cycle 2 bufs
            qc = [[ck_pool.tile([P, 512], bt, name=f"qc{h}_{j}", tag=f"qc{h}",
                                bufs=2)
                   for j in range(NQ)] for h in range(2)]
            kc = [[ck_pool.tile([P, 512], bt, name=f"kc{h}_{j}", tag=f"kc{h}_{j}")
                   for j in range(NQ)] for h in range(2)]
            vc = [[ck_pool.tile([P, 512], bt, name=f"vc{h}_{j}", tag=f"vc{h}_{j}")
                   for j in range(NQ)] for h in range(2)]

            # ---- input loads, priority-ordered per DMA ring ----------------
            # critical first: wqk (gpsimd ring) + hidden half-0 (sync/scalar/
            # vector rings round-robin), then wv, cos/sin/masks, hidden
            # half-1, and wo last (only needed by the output projection).
            # batched loads: few big access-pattern DMAs so the engine rings
            # are not clogged with issue slots; highest-priority data first,
            # interleaved in 4-e-chunk pieces so the first qkv chain can
            # start consuming early.
            wqk_big = []
            hid0_big = []
            for gi in range(4):
                wt = wts_pool.tile([P, 4, 512], bt, name=f"wqk_{gi}",
                                   tag="wqk", bufs=4)
                nc.gpsimd.dma_start(
                    wt[:], wqkT[512 * gi:512 * (gi + 1), :].rearrange(
                        "(o p) s -> p o s", p=P))
                wqk_big.append(wt)
                ht = hid_pool.tile([P, 4, 1024], bt, name=f"hid0_{gi}",
                                   tag="hid0", bufs=4)
                heng = nc.sync if gi % 2 == 0 else nc.scalar
                heng.dma_start(
                    ht[:], hiddenT[512 * gi:512 * (gi + 1), 0:1024].rearrange(
                        "(o p) s -> p o s", p=P))
                hid0_big.append(ht)
            wv_big = wts_pool.tile([P, NEC, 256], bt, name="wv", tag="wv")
            nc.gpsimd.dma_start(
                wv_big[:], wvT.rearrange("(o p) s -> p o s", p=P))
            nc.gpsimd.dma_start(cos_sb[:], cosT)
            nc.gpsimd.dma_start(sin_sb[:], sinT)
            nc.gpsimd.dma_start(mask_sb[:], masksT)
            hid1_big = hid_pool.tile([P, NEC, 1024], bt, name="hid1",
                                     tag="hid1")
            nc.sync.dma_start(
                hid1_big[:],
                hiddenT[:, 1024:2048].rearrange("(o p) s -> p o s", p=P))
            nc.gpsimd.dma_start(wo_sb[:], woutT.rearrange("p (o e) -> p o e",
                                                          o=NEC))

            wqk_sb = [wqk_big[e // 4][:, e % 4, :] for e in range(NEC)]
            wv_sb = [wv_big[:, e, :] for e in range(NEC)]

            oti_sb = []
            oti3_sb = []

            def hidsl(e, lo, width):  # [lo, lo+width) columns of chunk e
                if lo < 1024:
                    return hid0_big[e // 4][:, e % 4, lo:lo + width]
                return hid1_big[:, e, lo - 1024:lo - 1024 + width]

            def do_oproj(jj):
                qs = slice(512 * jj, 512 * (jj + 1))
                pps = [pp_pool.tile([P, 512], f32, name=f"pps{jj}_{b}",
                                    tag="gps") for b in range(2)]
                for fc in range(NEC):
                    for b in range(2):
                        nc.tensor.matmul(
                            pps[b][:], wo_sb[:, fc, b * P:(b + 1) * P],
                            oti_sb[jj][:, fc, :], start=(fc == 0),
                            stop=(fc == NEC - 1))
                for b in range(2):
                    ob = op_pool.tile([P, 512], bt, name=f"ob{jj}_{b}",
                                      tag="ob")
                    nc.scalar.copy(ob[:], pps[b][:])
                    nc.sync.dma_start(outT[b * P:(b + 1) * P, qs], ob[:])

            def do_oproj3():
                # chunk 3: even fc rows come from the head-0 AG, odd fc rows
                # from the head-1 AG; accumulate both phases into one PSUM
                # chain so the even half can run while head-1's AG is in
                # flight.
                qs = slice(1536, 2048)
                pps = [pp_pool.tile([P, 512], f32, name=f"pps3_{b}", tag="gps")
                       for b in range(2)]
                for phase in range(2):
                    for k in range(8):
                        fc = 2 * k + phase
                        for b in range(2):
                            nc.tensor.matmul(
                                pps[b][:], wo_sb[:, fc, b * P:(b + 1) * P],
                                oti3_sb[phase][:, k, :],
                                start=(phase == 0 and k == 0),
                                stop=(phase == 1 and k == 7))
                for b in range(2):
                    ob = op_pool.tile([P, 512], bt, name=f"ob3_{b}", tag="ob")
                    nc.scalar.copy(ob[:], pps[b][:])
                    nc.sync.dma_start(outT[b * P:(b + 1) * P, qs], ob[:])

            def do_qkv(j):
                qs = slice(512 * j, 512 * (j + 1))
                # ---- qkv chunk j: rot groups in 2 waves of 2 psum banks ----
                g01 = []
                for g in (0, 1):
                    gp = gps_pool.tile([P, 512], f32, name=f"gps{j}_{g}",
                                       tag="gps")
                    for e in range(NEC):
                        nc.tensor.matmul(
                            gp[:], wqk_sb[e][:, g * P:(g + 1) * P],
                            hidsl(e, 512 * j, 512), start=(e == 0),
                            stop=(e == NEC - 1))
                    gc = g01_pool.tile([P, 512], bt, name=f"g01_{j}_{g}",
                                       tag=f"g01_{g}")
                    nc.scalar.copy(gc[:], gp[:])
                    g01.append(gc)
                for g in (2, 3):
                    gp = gps_pool.tile([P, 512], f32, name=f"gps{j}_{g}",
                                       tag="gps")
                    for e in range(NEC):
                        nc.tensor.matmul(
                            gp[:], wqk_sb[e][:, g * P:(g + 1) * P],
                            hidsl(e, 512 * j, 512), start=(e == 0),
                            stop=(e == NEC - 1))
                    dst = qc if g == 2 else kc
                    nc.vector.tensor_copy(dst[0][j][64:128, :], gp[0:64, :])
                    nc.vector.tensor_copy(dst[1][j][64:128, :], gp[64:128, :])
                # v chunk j
                for st in range(4):
                    vp = gps_pool.tile([P, 256], f32, name=f"vps{j}_{st}",
                                       tag="gps")
                    for e in range(NEC):
                        nc.tensor.matmul(
                            vp[:], hidsl(e, 512 * j + st * P, P),
                            wv_sb[e][:], start=(e == 0), stop=(e == NEC - 1))
                    nc.vector.tensor_copy(vc[0][j][:, st * P:(st + 1) * P],
                                          vp[:, 0:P])
                    nc.vector.tensor_copy(vc[1][j][:, st * P:(st + 1) * P],
                                          vp[:, P:2 * P])
                # rope chunk j
                t0 = rtmp_pool.tile([P, 512], bt, name=f"t0_{j}", tag="t0")
                t1 = rtmp_pool.tile([P, 512], bt, name=f"t1_{j}", tag="t1")
                ta = rtmp_pool.tile([P, 512], bt, name=f"ta_{j}", tag="ta")
                tb = rtmp_pool.tile([P, 512], bt, name=f"tb_{j}", tag="tb")
                nc.vector.tensor_mul(t0[:], g01[0][:], cos_sb[:, qs])
                nc.vector.tensor_mul(t1[:], g01[1][:], sin_sb[:, qs])
                nc.vector.tensor_sub(ta[:], t0[:], t1[:])      # tops
                nc.vector.tensor_mul(t0[:], g01[1][:], cos_sb[:, qs])
                nc.vector.tensor_mul(t1[:], g01[0][:], sin_sb[:, qs])
                nc.vector.tensor_add(tb[:], t0[:], t1[:])      # bottoms
                for pc, dst in enumerate((qc[0], qc[1], kc[0], kc[1])):
                    ps = slice(32 * pc, 32 * (pc + 1))
                    nc.vector.tensor_copy(dst[j][0:32, :], ta[ps, :])
                    nc.vector.tensor_copy(dst[j][32:64, :], tb[ps, :])

            def do_attn(j, hi):
                nk = 4 * j + 4
                otp = ot_pool.tile([P, 512], f32, name=f"otp{hi}_{j}",
                                   tag="otp")
                dnp = dn_pool.tile([P, 512], f32, name=f"dnp{hi}_{j}",
                                   tag="dnp")
                for i in range(nk):
                    scp = sc_pool.tile([P, 512], f32, name=f"scp{hi}{j}{i}",
                                       tag="scp")
                    nc.tensor.matmul(
                        scp[:], kc[hi][i // 4][:, (i % 4) * P:
                                               (i % 4 + 1) * P],
                        qc[hi][j][:], start=True, stop=True)
                    pt = pt_pool.tile([P, 512], bt, name=f"pt{hi}{j}{i}",
                                      tag="pt")
                    nc.scalar.activation(
                        pt[:], scp[:], mybir.ActivationFunctionType.Exp,
                        scale=SCALE)
                    if i >= 4 * j:
                        off = 384 - 128 * (i - 4 * j)
                        nc.vector.tensor_mul(
                            pt[:], pt[:], mask_sb[:, off:off + 512])
                    nc.tensor.matmul(
                        otp[:], vc[hi][i // 4][:, (i % 4) * P:
                                               (i % 4 + 1) * P], pt[:],
                        start=(i == 0), stop=(i == nk - 1))
                    nc.tensor.matmul(
                        dnp[:], ones_sb[:], pt[:],
                        start=(i == 0), stop=(i == nk - 1))
                den_sb = oo_pool.tile([P, 512], f32, name=f"den{hi}_{j}",
                                      tag="den")
                nc.vector.reciprocal_approx_fast(den_sb[:], dnp[:])
                otn = oo_pool.tile([P, 512], bt, name=f"otn{hi}_{j}",
                                   tag="otn")
                nc.vector.tensor_mul(otn[:], otp[:], den_sb[:])
                if j < NQ - 1:
                    nc.sync.dma_start(ag_in[j][hi * P:(hi + 1) * P, :],
                                      otn[:])
                else:
                    nc.sync.dma_start(ag_in3[hi][:], otn[:])

            def prefetch_oti(j, pieces=1):
                # stage the gathered O^T; split into pieces so the consumer's
                # accumulation can start on piece 0 while later pieces stream
                oti = oi_pool.tile([P, NEC, 512], bt, name=f"oti{j}",
                                   tag="oti")
                w = NEC // pieces
                for k in range(pieces):
                    nc.gpsimd.dma_start(
                        oti[:, w * k:w * (k + 1), :],
                        ag_out[j][P * w * k:P * w * (k + 1), :].rearrange(
                            "(o p) s -> p o s", p=P))
                oti_sb.append(oti)

            def prefetch_oti3(hi, pieces=2):
                oti3 = oi3_pool.tile([P, 8, 512], bt, name=f"oti3{hi}",
                                     tag=f"oti3{hi}")
                w = 8 // pieces
                for k in range(pieces):
                    nc.gpsimd.dma_start(
                        oti3[:, w * k:w * (k + 1), :],
                        ag_out3[hi][P * w * k:P * w * (k + 1), :].rearrange(
                            "(o p) s -> p o s", p=P))
                oti3_sb.append(oti3)

            def fire_ag(j, prefetch=True):
                nc.gpsimd.collective_compute(
                    "AllGather",
                    mybir.AluOpType.bypass,
                    replica_groups=[list(range(NCORES))],
                    ins=[ag_in[j].opt()],
                    outs=[ag_out[j].opt()],
                )
                if prefetch:
                    prefetch_oti(j)

            def fire_ag3(hi, prefetch=True):
                nc.gpsimd.collective_compute(
                    "AllGather",
                    mybir.AluOpType.bypass,
                    replica_groups=[list(range(NCORES))],
                    ins=[ag_in3[hi].opt()],
                    outs=[ag_out3[hi].opt()],
                )
                if prefetch:
                    prefetch_oti3(hi)

            # schedule: chunks 0/1 pipelined; chunk 3 head 0's AG fires while
            # chunk 2's attention runs, chunk 2's AG while chunk 3 head 1
            # runs, so the CC engine stays busy and the final AG is the small
            # per-head one; all output projections run at the end, filling
            # the last AG's latency with PE work.
            do_qkv(0)
            do_attn(0, 0)
            do_attn(0, 1)
            fire_ag(0)
            do_qkv(1)
            do_attn(1, 0)
            do_attn(1, 1)
            fire_ag(1)
            do_qkv(2)
            do_qkv(3)
            do_attn(3, 0)
            fire_ag3(0)
            do_attn(3, 1)
            fire_ag3(1, prefetch=False)
            do_attn(2, 0)
            do_attn(2, 1)
            fire_ag(2, prefetch=False)
            # prefetches issued after both AG issues so a blocked DIRECT2D
            # (waiting on its AG's completion) never delays an AG issue
            prefetch_oti3(1)
            prefetch_oti(2, pieces=4)
            do_oproj(0)
            do_oproj(1)
            do_oproj3()
            do_oproj(2)


def _build_program():
    import concourse.bass as bass  # noqa: F401
    import concourse.mybir as mybir
    import concourse.tile as tile
    from concourse import bacc

    nc = bacc.Bacc("TRN2", target_bir_lowering=False, debug=False,
                   enable_asserts=False, num_devices=NCORES)
    f32 = mybir.dt.float32
    bt = mybir.dt.bfloat16
    hiddenT = nc.dram_tensor("hiddenT", [E, S], bt, kind="ExternalInput").ap()
    wqkT = nc.dram_tensor("wqkT", [E, 512], bt, kind="ExternalInput").ap()
    wvT = nc.dram_tensor("wvT", [E, 256], bt, kind="ExternalInput").ap()
    woutT = nc.dram_tensor("woutT", [P, NEC * 256], bt, kind="ExternalInput").ap()
    cosT = nc.dram_tensor("cosT", [P, S], bt, kind="ExternalInput").ap()
    sinT = nc.dram_tensor("sinT", [P, S], bt, kind="ExternalInput").ap()
    masks = nc.dram_tensor("masks", [P, 896], bt, kind="ExternalInput").ap()
    outT = nc.dram_tensor("outT", [2 * P, S], bt, kind="ExternalOutput").ap()

    with tile.TileContext(nc) as tc:
        _kernel_body(tc, outT, hiddenT, wqkT, wvT, woutT, cosT, sinT, masks)
    nc.compile()
    return nc


def get_program():
    if "nc" not in _CACHE:
        _CACHE["nc"] = _build_program()
    return _CACHE["nc"]


def _install_ntff_shim():
    """Provide antenv.axon_hooks (missing in this image) so trace=True works."""
    import sys
    import types
    try:
        import antenv.axon_hooks  # noqa: F401
        return
    except ImportError:
        pass
    import antenv
    mod = types.ModuleType("antenv.axon_hooks")
    mod._hook = None

    def set_axon_ntff_profile_hook(h):
        mod._hook = h

    def get_axon_ntff_profile_hook():
        return mod._hook

    mod.set_axon_ntff_profile_hook = set_axon_ntff_profile_hook
    mod.get_axon_ntff_profile_hook = get_axon_ntff_profile_hook
    sys.modules["antenv.axon_hooks"] = mod
    antenv.axon_hooks = mod
    try:
        from trn_agent_boot.trn_boot import _ntff_profile_via_ctypes
        hook = _ntff_profile_via_ctypes("/opt/axon/libaxon_pjrt.so")
        if hook is not None:
            mod._hook = hook
    except Exception:
        pass


def run(inputs, trace=False):
    """Run on the 8 NeuronCores; returns (out [1,S,E], BassKernelResults)."""
    from concourse import bass_utils

    if trace:
        _install_ntff_shim()
    nc = get_program()
    in_maps = _build_in_maps(inputs["hidden_states"], inputs["w_qkv"],
                             inputs["w_out"])
    res = bass_utils.run_bass_kernel_spmd(
        nc, in_maps, core_ids=list(range(NCORES)), trace=trace)
    outT = np.concatenate(
        [np.asarray(res.results[c]["outT"], dtype=np.float32)
         for c in range(NCORES)], axis=0)  # [E, S]
    out = np.ascontiguousarray(outT.T).reshape(1, S, E).astype(np.float32)
    return out, res


def kernel(hidden_states, w_qkv, w_out):
    out, _ = run({"hidden_states": hidden_states, "w_qkv": w_qkv,
                  "w_out": w_out})
    return out


# revision 19
# speedup vs baseline: 1.0027x; 1.0027x over previous
"""CodeGen attention block (B=1, S=2048, E=2048, H=16, D=128, rot=64) on 8 TRN2
NeuronCores.

Sharding: tensor-parallel over heads (2 heads/core). Each core computes its
heads' q/k in transposed [d, s] layout (partial rotary applied via a host-side
even/odd channel permutation folded into the qkv weights, which turns the
interleaved rotation into a rotate-half form on contiguous partition blocks),
v in [s, d] layout, causal softmax attention entirely on-chip (scores
transposed [k, q] so the P·V matmul needs no transposes; softmax denominators
via a ones-stationary matmul), then AllGathers the per-core attention output
O^T [256, 2048] and computes a 256-column slice of the output projection.
Host assembles the 8 slices and transposes.

All PE-facing tensors are bf16 (full PE speed, half the DMA); accumulation
is fp32 in PSUM. The AllGather is split per-s-chunk (and per-head for the
last chunk) so collectives fire as soon as the producing attention work
finishes, overlapping comm with attention; the output projection for chunk j
is interleaved into the main loop after chunk j+2's attention so PE never
waits on a collective except at the very tail.
"""

import numpy as np

H, D, ROT, MP = 16, 128, 64, 4
S, E = 2048, 2048
NCORES = 8
P = 128
NQ = 4            # 512-wide q chunks
NKT = S // P      # 16 k tiles
NEC = E // P      # 16 contraction chunks
SCALE = float(1.0 / np.sqrt(np.float64(D)))

_CACHE = {}


# ----------------------------------------------------------------------------
# host-side input prep
# ----------------------------------------------------------------------------

def _head_rows(h):
    g, j = h // 4, h % 4
    base = g * (3 * 512)
    q = np.arange(base + j * 128, base + (j + 1) * 128)
    v = np.arange(base + 512 + j * 128, base + 512 + (j + 1) * 128)
    k = np.arange(base + 1024 + j * 128, base + 1024 + (j + 1) * 128)
    return q, v, k


def _prep_core_weights(c, w_qkv, w_out):
    h0, h1 = 2 * c, 2 * c + 1
    top = np.arange(0, ROT, 2)
    bot = np.arange(1, ROT, 2)
    wq, wk, wv = {}, {}, {}
    for h in (h0, h1):
        qr, vr, kr = _head_rows(h)
        wq[h], wv[h], wk[h] = w_qkv[qr], w_qkv[vr], w_qkv[kr]
    G0 = np.concatenate([wq[h0][top], wq[h1][top], wk[h0][top], wk[h1][top]], 0)
    G1 = np.concatenate([wq[h0][bot], wq[h1][bot], wk[h0][bot], wk[h1][bot]], 0)
    G2 = np.concatenate([wq[h0][ROT:], wq[h1][ROT:]], 0)
    G3 = np.concatenate([wk[h0][ROT:], wk[h1][ROT:]], 0)
    wqkT = np.ascontiguousarray(
        np.concatenate([G0, G1, G2, G3], 0).T, dtype=np.float32)       # [E, 512]
    wvT = np.ascontiguousarray(
        np.concatenate([wv[h0], wv[h1]], 0).T, dtype=np.float32)       # [E, 256]
    woutT = np.ascontiguousarray(
        w_out[256 * c:256 * (c + 1), :].T, dtype=np.float32)           # [E, 256]
    # pre-pack for a fully contiguous [128, 16, 256] SBUF load
    woutT = np.ascontiguousarray(
        woutT.reshape(16, 128, 256).transpose(1, 0, 2)).reshape(128, 16 * 256)
    return wqkT, wvT, woutT


def _cos_sin():
    inv_freq = 1.0 / (10000.0 ** (np.arange(0, ROT, 2, dtype=np.float32) / ROT))
    ang = np.arange(S, dtype=np.float32)[:, None] * inv_freq[None, :]
    cosb = np.cos(ang).T.astype(np.float32)        # [32, S]
    sinb = np.sin(ang).T.astype(np.float32)
    return (np.ascontiguousarray(np.tile(cosb, (4, 1))),
            np.ascontiguousarray(np.tile(sinb, (4, 1))))               # [128, S]


def _mask_tiles():
    # single shifted mask: M[p, x] = (p <= x - 384); the diagonal-offset-mm
    # mask is the 512-wide view starting at column 384 - 128*mm
    pp = np.arange(128)[:, None]
    xx = np.arange(896)[None, :]
    return (xx >= pp + 384).astype(np.float32)


def _build_in_maps(hidden_states, w_qkv, w_out):
    import ml_dtypes
    bf16 = ml_dtypes.bfloat16
    hiddenT = np.ascontiguousarray(
        np.asarray(hidden_states, np.float32).reshape(S, E).T).astype(bf16)
    COS, SIN = _cos_sin()
    COS, SIN = COS.astype(bf16), SIN.astype(bf16)
    masks = _mask_tiles().astype(bf16)
    in_maps = []
    for c in range(NCORES):
        wqkT, wvT, woutT = _prep_core_weights(
            c, np.asarray(w_qkv, np.float32), np.asarray(w_out, np.float32))
        in_maps.append({
            "hiddenT": hiddenT,
            "wqkT": wqkT.astype(bf16),
            "wvT": wvT.astype(bf16),
            "woutT": woutT.astype(bf16),
            "cosT": COS,
            "sinT": SIN,
            "masks": masks,
        })
    return in_maps


# ----------------------------------------------------------------------------
# device program
# ----------------------------------------------------------------------------

def _kernel_body(tc, outT, hiddenT, wqkT, wvT, woutT, cosT, sinT, masksT):
    import concourse.mybir as mybir
    from contextlib import ExitStack

    nc = tc.nc
    f32 = mybir.dt.float32
    bt = mybir.dt.bfloat16

    with ExitStack() as ctx:
        const = ctx.enter_context(tc.tile_pool(name="const", bufs=1))
        mask_sb = const.tile([P, 896], bt, name="mask_sb")
        ones_sb = const.tile([P, P], bt, name="ones_sb")
        wo_sb = const.tile([P, NEC, 256], bt, name="wo_sb")
        nc.vector.memset(ones_sb[:], 1.0)

        dram = ctx.enter_context(tc.tile_pool(name="dram", bufs=1, space="DRAM"))
        ag_in = [dram.tile([2 * P, 512], bt, name=f"ag_in{j}", tag=f"ag_in{j}")
                 for j in range(NQ - 1)]
        ag_out = [dram.tile([E, 512], bt, name=f"ag_out{j}", tag=f"ag_out{j}",
                            addr_space="Shared") for j in range(NQ - 1)]
        # last chunk: per-head collectives so comm starts as soon as head 0
        # finishes and the tail only waits on head 1's small AG
        ag_in3 = [dram.tile([P, 512], bt, name=f"ag_in3{h}", tag=f"ag_in3{h}")
                  for h in range(2)]
        ag_out3 = [dram.tile([8 * P, 512], bt, name=f"ag_out3{h}",
                             tag=f"ag_out3{h}", addr_space="Shared")
                   for h in range(2)]

        # --- phase 1+2 interleaved: per-chunk qkv -> attention -> AG --------
        with tc.tile_pool(name="ph1c", bufs=1) as ph1c_pool, \
             tc.tile_pool(name="chunk", bufs=1) as ck_pool, \
             tc.tile_pool(name="hid", bufs=1) as hid_pool, \
             tc.tile_pool(name="wts", bufs=1) as wts_pool, \
             tc.tile_pool(name="g01c", bufs=1) as g01_pool, \
             tc.tile_pool(name="rtmp", bufs=1) as rtmp_pool, \
             tc.tile_pool(name="pt", bufs=6) as pt_pool, \
             tc.tile_pool(name="oout", bufs=2) as oo_pool, \
             tc.tile_pool(name="otin", bufs=2) as oi_pool, \
             tc.tile_pool(name="otin3", bufs=1) as oi3_pool, \
             tc.tile_pool(name="oprj", bufs=2) as op_pool, \
             tc.tile_pool(name="gps", bufs=2, space="PSUM") as gps_pool, \
             tc.tile_pool(name="scps", bufs=2, space="PSUM") as sc_pool, \
             tc.tile_pool(name="otps", bufs=2, space="PSUM") as ot_pool, \
             tc.tile_pool(name="dnps", bufs=2, space="PSUM") as dn_pool:

            pp_pool = gps_pool  # oproj PSUM reuses the (idle-by-then) qkv banks

            cos_sb = ph1c_pool.tile([P, S], bt, name="cos_sb", tag="cos_sb")
            sin_sb = ph1c_pool.tile([P, S], bt, name="sin_sb", tag="sin_sb")

            # per-chunk activations: q/k transposed [d, 512]; v [k-in-tile, 4*128]
            # q is only live from its qkv until its attention -> cycle 2 bufs
            qc = [[ck_pool.tile([P, 512], bt, name=f"qc{h}_{j}", tag=f"qc{h}",
                                bufs=2)
                   for j in range(NQ)] for h in range(2)]
            kc = [[ck_pool.tile([P, 512], bt, name=f"kc{h}_{j}", tag=f"kc{h}_{j}")
                   for j in range(NQ)] for h in range(2)]
            vc = [[ck_pool.tile([P, 512], bt, name=f"vc{h}_{j}", tag=f"vc{h}_{j}")
                   for j in range(NQ)] for h in range(2)]

            # ---- input loads, priority-ordered per DMA ring ----------------
            # critical first: wqk (gpsimd ring) + hidden half-0 (sync/scalar/
            # vector rings round-robin), then wv, cos/sin/masks, hidden
            # half-1, and wo last (only needed by the output projection).
            # input loads, priority-ordered per DMA ring: wqk (gpsimd) and
            # hidden half-0 (sync/scalar alternating) first, then wv/rope
            # tables/masks, hidden half-1 (all on sync so the scalar ring is
            # free for the qkv-phase copies), and wo last.
            wqk_sb = []
            wv_sb = []
            hid_sb = [[None, None] for _ in range(NEC)]
            for e in range(NEC):
                wq_tile = wts_pool.tile([P, 512], bt, name=f"wqk_{e}",
                                        tag="wqk", bufs=NEC)
                nc.gpsimd.dma_start(wq_tile[:], wqkT[e * P:(e + 1) * P, :])
                wqk_sb.append(wq_tile)
                ht = hid_pool.tile([P, 1024], bt, name=f"hid_{e}_0",
                                   tag="hid", bufs=2 * NEC)
                heng = nc.sync if e % 2 == 0 else nc.scalar
                heng.dma_start(ht[:], hiddenT[e * P:(e + 1) * P, 0:1024])
                hid_sb[e][0] = ht
            for e in range(NEC):
                wv_tile = wts_pool.tile([P, 256], bt, name=f"wv_{e}",
                                        tag="wv", bufs=NEC)
                nc.gpsimd.dma_start(wv_tile[:], wvT[e * P:(e + 1) * P, :])
                wv_sb.append(wv_tile)
            nc.gpsimd.dma_start(cos_sb[:], cosT)
            nc.gpsimd.dma_start(sin_sb[:], sinT)
            nc.gpsimd.dma_start(mask_sb[:], masksT)
            for e in range(NEC):
                ht = hid_pool.tile([P, 1024], bt, name=f"hid_{e}_1",
                                   tag="hid", bufs=2 * NEC)
                nc.sync.dma_start(ht[:], hiddenT[e * P:(e + 1) * P,
                                                 1024:2048])
                hid_sb[e][1] = ht
            nc.gpsimd.dma_start(wo_sb[:], woutT.rearrange("p (o e) -> p o e",
                                                          o=NEC))

            oti_sb = []
            oti3_sb = []

            def hidsl(e, lo, width):  # [lo, lo+width) columns of chunk e
                half = hid_sb[e][lo // 1024]
                off = lo % 1024
                return half[:, off:off + width]

            def do_oproj(jj):
                qs = slice(512 * jj, 512 * (jj + 1))
                pps = [pp_pool.tile([P, 512], f32, name=f"pps{jj}_{b}",
                                    tag="gps") for b in range(2)]
                for fc in range(NEC):
                    for b in range(2):
                        nc.tensor.matmul(
                            pps[b][:], wo_sb[:, fc, b * P:(b + 1) * P],
                            oti_sb[jj][:, fc, :], start=(fc == 0),
                            stop=(fc == NEC - 1))
                for b in range(2):
                    ob = op_pool.tile([P, 512], bt, name=f"ob{jj}_{b}",
                                      tag="ob")
                    nc.scalar.copy(ob[:], pps[b][:])
                    nc.sync.dma_start(outT[b * P:(b + 1) * P, qs], ob[:])

            def do_oproj3():
                # chunk 3: even fc rows come from the head-0 AG, odd fc rows
                # from the head-1 AG; accumulate both phases into one PSUM
                # chain so the even half can run while head-1's AG is in
                # flight.
                qs = slice(1536, 2048)
                pps = [pp_pool.tile([P, 512], f32, name=f"pps3_{b}", tag="gps")
                       for b in range(2)]
                for phase in range(2):
                    for k in range(8):
                        fc = 2 * k + phase
                        for b in range(2):
                            nc.tensor.matmul(
                                pps[b][:], wo_sb[:, fc, b * P:(b + 1) * P],
                                oti3_sb[phase][:, k, :],
                                start=(phase == 0 and k == 0),
                                stop=(phase == 1 and k == 7))
                for b in range(2):
                    ob = op_pool.tile([P, 512], bt, name=f"ob3_{b}", tag="ob")
                    nc.scalar.copy(ob[:], pps[b][:])
                    nc.sync.dma_start(outT[b * P:(b + 1) * P, qs], ob[:])

            def do_qkv(j):
                qs = slice(512 * j, 512 * (j + 1))
                # ---- qkv chunk j: rot groups in 2 waves of 2 psum banks ----
                g01 = []
                for g in (0, 1):
                    gp = gps_pool.tile([P, 512], f32, name=f"gps{j}_{g}",
                                       tag="gps")
                    for e in range(NEC):
                        nc.tensor.matmul(
                            gp[:], wqk_sb[e][:, g * P:(g + 1) * P],
                            hidsl(e, 512 * j, 512), start=(e == 0),
                            stop=(e == NEC - 1))
                    gc = g01_pool.tile([P, 512], bt, name=f"g01_{j}_{g}",
                                       tag=f"g01_{g}")
                    nc.scalar.copy(gc[:], gp[:])
                    g01.append(gc)
                for g in (2, 3):
                    gp = gps_pool.tile([P, 512], f32, name=f"gps{j}_{g}",
                                       tag="gps")
                    for e in range(NEC):
                        nc.tensor.matmul(
                            gp[:], wqk_sb[e][:, g * P:(g + 1) * P],
                            hidsl(e, 512 * j, 512), start=(e == 0),
                            stop=(e == NEC - 1))
                    dst = qc if g == 2 else kc
                    nc.vector.tensor_copy(dst[0][j][64:128, :], gp[0:64, :])
                    nc.vector.tensor_copy(dst[1][j][64:128, :], gp[64:128, :])
                # v chunk j
                for st in range(4):
                    vp = gps_pool.tile([P, 256], f32, name=f"vps{j}_{st}",
                                       tag="gps")
                    for e in range(NEC):
                        nc.tensor.matmul(
                            vp[:], hidsl(e, 512 * j + st * P, P),
                            wv_sb[e][:], start=(e == 0), stop=(e == NEC - 1))
                    nc.vector.tensor_copy(vc[0][j][:, st * P:(st + 1) * P],
                                          vp[:, 0:P])
                    nc.vector.tensor_copy(vc[1][j][:, st * P:(st + 1) * P],
                                          vp[:, P:2 * P])
                # rope chunk j
                t0 = rtmp_pool.tile([P, 512], bt, name=f"t0_{j}", tag="t0")
                t1 = rtmp_pool.tile([P, 512], bt, name=f"t1_{j}", tag="t1")
                ta = rtmp_pool.tile([P, 512], bt, name=f"ta_{j}", tag="ta")
                tb = rtmp_pool.tile([P, 512], bt, name=f"tb_{j}", tag="tb")
                nc.vector.tensor_mul(t0[:], g01[0][:], cos_sb[:, qs])
                nc.vector.tensor_mul(t1[:], g01[1][:], sin_sb[:, qs])
                nc.vector.tensor_sub(ta[:], t0[:], t1[:])      # tops
                nc.vector.tensor_mul(t0[:], g01[1][:], cos_sb[:, qs])
                nc.vector.tensor_mul(t1[:], g01[0][:], sin_sb[:, qs])
                nc.vector.tensor_add(tb[:], t0[:], t1[:])      # bottoms
                for pc, dst in enumerate((qc[0], qc[1], kc[0], kc[1])):
                    ps = slice(32 * pc, 32 * (pc + 1))
                    nc.vector.tensor_copy(dst[j][0:32, :], ta[ps, :])
                    nc.vector.tensor_copy(dst[j][32:64, :], tb[ps, :])

            def do_attn(j, hi):
                nk = 4 * j + 4
                otp = ot_pool.tile([P, 512], f32, name=f"otp{hi}_{j}",
                                   tag="otp")
                dnp = dn_pool.tile([P, 512], f32, name=f"dnp{hi}_{j}",
                                   tag="dnp")
                for i in range(nk):
                    scp = sc_pool.tile([P, 512], f32, name=f"scp{hi}{j}{i}",
                                       tag="scp")
                    nc.tensor.matmul(
                        scp[:], kc[hi][i // 4][:, (i % 4) * P:
                                               (i % 4 + 1) * P],
                        qc[hi][j][:], start=True, stop=True)
                    pt = pt_pool.tile([P, 512], bt, name=f"pt{hi}{j}{i}",
                                      tag="pt")
                    nc.scalar.activation(
                        pt[:], scp[:], mybir.ActivationFunctionType.Exp,
                        scale=SCALE)
                    if i >= 4 * j:
                        off = 384 - 128 * (i - 4 * j)
                        nc.vector.tensor_mul(
                            pt[:], pt[:], mask_sb[:, off:off + 512])
                    nc.tensor.matmul(
                        otp[:], vc[hi][i // 4][:, (i % 4) * P:
                                               (i % 4 + 1) * P], pt[:],
                        start=(i == 0), stop=(i == nk - 1))
                    nc.tensor.matmul(
                        dnp[:], ones_sb[:], pt[:],
                        start=(i == 0), stop=(i == nk - 1))
                den_sb = oo_pool.tile([P, 512], f32, name=f"den{hi}_{j}",
                                      tag="den")
                nc.vector.reciprocal_approx_fast(den_sb[:], dnp[:])
                otn = oo_pool.tile([P, 512], bt, name=f"otn{hi}_{j}",
                                   tag="otn")
                nc.vector.tensor_mul(otn[:], otp[:], den_sb[:])
                if j < NQ - 1:
                    nc.sync.dma_start(ag_in[j][hi * P:(hi + 1) * P, :],
                                      otn[:])
                else:
                    nc.sync.dma_start(ag_in3[hi][:], otn[:])

            def prefetch_oti(j, pieces=1):
                # stage the gathered O^T; split into pieces so the consumer's
                # accumulation can start on piece 0 while later pieces stream
                oti = oi_pool.tile([P, NEC, 512], bt, name=f"oti{j}",
                                   tag="oti")
                w = NEC // pieces
                for k in range(pieces):
                    nc.gpsimd.dma_start(
                        oti[:, w * k:w * (k + 1), :],
                        ag_out[j][P * w * k:P * w * (k + 1), :].rearrange(
                            "(o p) s -> p o s", p=P))
                oti_sb.append(oti)

            def prefetch_oti3(hi, pieces=2):
                oti3 = oi3_pool.tile([P, 8, 512], bt, name=f"oti3{hi}",
                                     tag=f"oti3{hi}")
                w = 8 // pieces
                for k in range(pieces):
                    nc.gpsimd.dma_start(
                        oti3[:, w * k:w * (k + 1), :],
                        ag_out3[hi][P * w * k:P * w * (k + 1), :].rearrange(
                            "(o p) s -> p o s", p=P))
                oti3_sb.append(oti3)

            def fire_ag(j, prefetch=True):
                nc.gpsimd.collective_compute(
                    "AllGather",
                    mybir.AluOpType.bypass,
                    replica_groups=[list(range(NCORES))],
                    ins=[ag_in[j].opt()],
                    outs=[ag_out[j].opt()],
                )
                if prefetch:
                    prefetch_oti(j)

            def fire_ag3(hi, prefetch=True):
                nc.gpsimd.collective_compute(
                    "AllGather",
                    mybir.AluOpType.bypass,
                    replica_groups=[list(range(NCORES))],
                    ins=[ag_in3[hi].opt()],
                    outs=[ag_out3[hi].opt()],
                )
                if prefetch:
                    prefetch_oti3(hi)

            # schedule: chunks 0/1 pipelined; chunk 3 head 0's AG fires while
            # chunk 2's attention runs, chunk 2's AG while chunk 3 head 1
            # runs, so the CC engine stays busy and the final AG is the small
            # per-head one; all output projections run at the end, filling
            # the last AG's latency with PE work.
            do_qkv(0)
            do_attn(0, 0)
            do_attn(0, 1)
            fire_ag(0)
            do_qkv(1)
            do_attn(1, 0)
            do_attn(1, 1)
            fire_ag(1)
            do_qkv(2)
            do_qkv(3)
            do_attn(3, 0)
            fire_ag3(0)
            do_attn(2, 0)
            do_attn(2, 1)
            fire_ag(2, prefetch=False)
            do_attn(3, 1)
            fire_ag3(1, prefetch=False)
            # prefetches issued after both AG issues so a blocked DIRECT2D
            # (waiting on its AG's completion) never delays an AG issue;
            # split into pieces so the consuming oproj pipelines with the DMA
            prefetch_oti(2, pieces=4)
            prefetch_oti3(1, pieces=2)
            do_oproj(0)
            do_oproj(1)
            do_oproj(2)
            do_oproj3()


def _build_program():
    import concourse.bass as bass  # noqa: F401
    import concourse.mybir as mybir
    import concourse.tile as tile
    from concourse import bacc

    nc = bacc.Bacc("TRN2", target_bir_lowering=False, debug=False,
                   enable_asserts=False, num_devices=NCORES)
    f32 = mybir.dt.float32
    bt = mybir.dt.bfloat16
    hiddenT = nc.dram_tensor("hiddenT", [E, S], bt, kind="ExternalInput").ap()
    wqkT = nc.dram_tensor("wqkT", [E, 512], bt, kind="ExternalInput").ap()
    wvT = nc.dram_tensor("wvT", [E, 256], bt, kind="ExternalInput").ap()
    woutT = nc.dram_tensor("woutT", [P, NEC * 256], bt, kind="ExternalInput").ap()
    cosT = nc.dram_tensor("cosT", [P, S], bt, kind="ExternalInput").ap()
    sinT = nc.dram_tensor("sinT", [P, S], bt, kind="ExternalInput").ap()
    masks = nc.dram_tensor("masks", [P, 896], bt, kind="ExternalInput").ap()
    outT = nc.dram_tensor("outT", [2 * P, S], bt, kind="ExternalOutput").ap()

    with tile.TileContext(nc) as tc:
        _kernel_body(tc, outT, hiddenT, wqkT, wvT, woutT, cosT, sinT, masks)
    nc.compile()
    return nc


def get_program():
    if "nc" not in _CACHE:
        _CACHE["nc"] = _build_program()
    return _CACHE["nc"]


def _install_ntff_shim():
    """Provide antenv.axon_hooks (missing in this image) so trace=True works."""
    import sys
    import types
    try:
        import antenv.axon_hooks  # noqa: F401
        return
    except ImportError:
        pass
    import antenv
    mod = types.ModuleType("antenv.axon_hooks")
    mod._hook = None

    def set_axon_ntff_profile_hook(h):
        mod._hook = h

    def get_axon_ntff_profile_hook():
        return mod._hook

    mod.set_axon_ntff_profile_hook = set_axon_ntff_profile_hook
    mod.get_axon_ntff_profile_hook = get_axon_ntff_profile_hook
    sys.modules["antenv.axon_hooks"] = mod
    antenv.axon_hooks = mod
    try:
        from trn_agent_boot.trn_boot import _ntff_profile_via_ctypes
        hook = _ntff_profile_via_ctypes("/opt/axon/libaxon_pjrt.so")
        if hook is not None:
            mod._hook = hook
    except Exception:
        pass


def run(inputs, trace=False):
    """Run on the 8 NeuronCores; returns (out [1,S,E], BassKernelResults)."""
    from concourse import bass_utils

    if trace:
        _install_ntff_shim()
    nc = get_program()
    in_maps = _build_in_maps(inputs["hidden_states"], inputs["w_qkv"],
                             inputs["w_out"])
    res = bass_utils.run_bass_kernel_spmd(
        nc, in_maps, core_ids=list(range(NCORES)), trace=trace)
    outT = np.concatenate(
        [np.asarray(res.results[c]["outT"], dtype=np.float32)
         for c in range(NCORES)], axis=0)  # [E, S]
    out = np.ascontiguousarray(outT.T).reshape(1, S, E).astype(np.float32)
    return out, res


def kernel(hidden_states, w_qkv, w_out):
    out, _ = run({"hidden_states": hidden_states, "w_qkv": w_qkv,
                  "w_out": w_out})
    return out


# revision 20
# speedup vs baseline: 1.0135x; 1.0108x over previous
"""CodeGen attention block (B=1, S=2048, E=2048, H=16, D=128, rot=64) on 8 TRN2
NeuronCores.

Sharding: tensor-parallel over heads (2 heads/core). Each core computes its
heads' q/k in transposed [d, s] layout (partial rotary applied via a host-side
even/odd channel permutation folded into the qkv weights, which turns the
interleaved rotation into a rotate-half form on contiguous partition blocks),
v in [s, d] layout, causal softmax attention entirely on-chip (scores
transposed [k, q] so the P·V matmul needs no transposes; softmax denominators
via a ones-stationary matmul), then AllGathers the per-core attention output
O^T [256, 2048] and computes a 256-column slice of the output projection.
Host assembles the 8 slices and transposes.

All PE-facing tensors are bf16 (full PE speed, half the DMA); accumulation
is fp32 in PSUM. The AllGather is split per-s-chunk (and per-head for the
last chunk) so collectives fire as soon as the producing attention work
finishes, overlapping comm with attention; the output projection for chunk j
is interleaved into the main loop after chunk j+2's attention so PE never
waits on a collective except at the very tail.
"""

import numpy as np

H, D, ROT, MP = 16, 128, 64, 4
S, E = 2048, 2048
NCORES = 8
P = 128
NQ = 4            # 512-wide q chunks
NKT = S // P      # 16 k tiles
NEC = E // P      # 16 contraction chunks
SCALE = float(1.0 / np.sqrt(np.float64(D)))

_CACHE = {}


# ----------------------------------------------------------------------------
# host-side input prep
# ----------------------------------------------------------------------------

def _head_rows(h):
    g, j = h // 4, h % 4
    base = g * (3 * 512)
    q = np.arange(base + j * 128, base + (j + 1) * 128)
    v = np.arange(base + 512 + j * 128, base + 512 + (j + 1) * 128)
    k = np.arange(base + 1024 + j * 128, base + 1024 + (j + 1) * 128)
    return q, v, k


def _prep_core_weights(c, w_qkv, w_out):
    h0, h1 = 2 * c, 2 * c + 1
    top = np.arange(0, ROT, 2)
    bot = np.arange(1, ROT, 2)
    wq, wk, wv = {}, {}, {}
    for h in (h0, h1):
        qr, vr, kr = _head_rows(h)
        wq[h], wv[h], wk[h] = w_qkv[qr], w_qkv[vr], w_qkv[kr]
    G0 = np.concatenate([wq[h0][top], wq[h1][top], wk[h0][top], wk[h1][top]], 0)
    G1 = np.concatenate([wq[h0][bot], wq[h1][bot], wk[h0][bot], wk[h1][bot]], 0)
    G2 = np.concatenate([wq[h0][ROT:], wq[h1][ROT:]], 0)
    G3 = np.concatenate([wk[h0][ROT:], wk[h1][ROT:]], 0)
    wqkT = np.ascontiguousarray(
        np.concatenate([G0, G1, G2, G3], 0).T, dtype=np.float32)       # [E, 512]
    wvT = np.ascontiguousarray(
        np.concatenate([wv[h0], wv[h1]], 0).T, dtype=np.float32)       # [E, 256]
    woutT = np.ascontiguousarray(
        w_out[256 * c:256 * (c + 1), :].T, dtype=np.float32)           # [E, 256]
    # pre-pack for a fully contiguous [128, 16, 256] SBUF load
    woutT = np.ascontiguousarray(
        woutT.reshape(16, 128, 256).transpose(1, 0, 2)).reshape(128, 16 * 256)
    return wqkT, wvT, woutT


def _cos_sin():
    inv_freq = 1.0 / (10000.0 ** (np.arange(0, ROT, 2, dtype=np.float32) / ROT))
    ang = np.arange(S, dtype=np.float32)[:, None] * inv_freq[None, :]
    cosb = np.cos(ang).T.astype(np.float32)        # [32, S]
    sinb = np.sin(ang).T.astype(np.float32)
    return (np.ascontiguousarray(np.tile(cosb, (4, 1))),
            np.ascontiguousarray(np.tile(sinb, (4, 1))))               # [128, S]


def _mask_tiles():
    # single shifted mask: M[p, x] = (p <= x - 384); the diagonal-offset-mm
    # mask is the 512-wide view starting at column 384 - 128*mm
    pp = np.arange(128)[:, None]
    xx = np.arange(896)[None, :]
    return (xx >= pp + 384).astype(np.float32)


def _build_in_maps(hidden_states, w_qkv, w_out):
    import ml_dtypes
    bf16 = ml_dtypes.bfloat16
    hiddenT = np.ascontiguousarray(
        np.asarray(hidden_states, np.float32).reshape(S, E).T).astype(bf16)
    COS, SIN = _cos_sin()
    COS, SIN = COS.astype(bf16), SIN.astype(bf16)
    masks = _mask_tiles().astype(bf16)
    in_maps = []
    for c in range(NCORES):
        wqkT, wvT, woutT = _prep_core_weights(
            c, np.asarray(w_qkv, np.float32), np.asarray(w_out, np.float32))
        in_maps.append({
            "hiddenT": hiddenT,
            "wqkT": wqkT.astype(bf16),
            "wvT": wvT.astype(bf16),
            "woutT": woutT.astype(bf16),
            "cosT": COS,
            "sinT": SIN,
            "masks": masks,
        })
    return in_maps


# ----------------------------------------------------------------------------
# device program
# ----------------------------------------------------------------------------

def _kernel_body(tc, outT, hiddenT, wqkT, wvT, woutT, cosT, sinT, masksT):
    import concourse.mybir as mybir
    from contextlib import ExitStack

    nc = tc.nc
    f32 = mybir.dt.float32
    bt = mybir.dt.bfloat16

    with ExitStack() as ctx:
        const = ctx.enter_context(tc.tile_pool(name="const", bufs=1))
        mask_sb = const.tile([P, 896], bt, name="mask_sb")
        ones_sb = const.tile([P, P], bt, name="ones_sb")
        wo_sb = const.tile([P, NEC, 256], bt, name="wo_sb")
        nc.vector.memset(ones_sb[:], 1.0)

        dram = ctx.enter_context(tc.tile_pool(name="dram", bufs=1, space="DRAM"))
        ag_in = [dram.tile([2 * P, 512], bt, name=f"ag_in{j}", tag=f"ag_in{j}")
                 for j in range(NQ - 1)]
        ag_out = [dram.tile([E, 512], bt, name=f"ag_out{j}", tag=f"ag_out{j}",
                            addr_space="Shared") for j in range(NQ - 1)]
        # last chunk: per-head collectives so comm starts as soon as head 0
        # finishes and the tail only waits on head 1's small AG
        ag_in3 = [dram.tile([P, 512], bt, name=f"ag_in3{h}", tag=f"ag_in3{h}")
                  for h in range(2)]
        ag_out3 = [dram.tile([8 * P, 512], bt, name=f"ag_out3{h}",
                             tag=f"ag_out3{h}", addr_space="Shared")
                   for h in range(2)]

        # --- phase 1+2 interleaved: per-chunk qkv -> attention -> AG --------
        with tc.tile_pool(name="ph1c", bufs=1) as ph1c_pool, \
             tc.tile_pool(name="chunk", bufs=1) as ck_pool, \
             tc.tile_pool(name="hid", bufs=2 * NEC) as hid_pool, \
             tc.tile_pool(name="wts", bufs=NEC) as wts_pool, \
             tc.tile_pool(name="g01c", bufs=1) as g01_pool, \
             tc.tile_pool(name="rtmp", bufs=1) as rtmp_pool, \
             tc.tile_pool(name="pt", bufs=6) as pt_pool, \
             tc.tile_pool(name="oout", bufs=2) as oo_pool, \
             tc.tile_pool(name="otin", bufs=2) as oi_pool, \
             tc.tile_pool(name="otin3", bufs=1) as oi3_pool, \
             tc.tile_pool(name="oprj", bufs=2) as op_pool, \
             tc.tile_pool(name="gps", bufs=2, space="PSUM") as gps_pool, \
             tc.tile_pool(name="scps", bufs=2, space="PSUM") as sc_pool, \
             tc.tile_pool(name="otps", bufs=2, space="PSUM") as ot_pool, \
             tc.tile_pool(name="dnps", bufs=2, space="PSUM") as dn_pool:

            pp_pool = gps_pool  # oproj PSUM reuses the (idle-by-then) qkv banks

            cos_sb = ph1c_pool.tile([P, S], bt, name="cos_sb", tag="cos_sb")
            sin_sb = ph1c_pool.tile([P, S], bt, name="sin_sb", tag="sin_sb")

            # per-chunk activations: q/k transposed [d, 512]; v [k-in-tile, 4*128]
            # q is only live from its qkv until its attention -> cycle 2 bufs
            qc = [[ck_pool.tile([P, 512], bt, name=f"qc{h}_{j}", tag=f"qc{h}",
                                bufs=2)
                   for j in range(NQ)] for h in range(2)]
            kc = [[ck_pool.tile([P, 512], bt, name=f"kc{h}_{j}", tag=f"kc{h}_{j}")
                   for j in range(NQ)] for h in range(2)]
            vc = [[ck_pool.tile([P, 512], bt, name=f"vc{h}_{j}", tag=f"vc{h}_{j}")
                   for j in range(NQ)] for h in range(2)]

            # ---- input loads, priority-ordered per DMA ring ----------------
            # critical first: wqk (gpsimd ring) + hidden half-0 (sync/scalar/
            # vector rings round-robin), then wv, cos/sin/masks, hidden
            # half-1, and wo last (only needed by the output projection).
            wqk_sb = []
            wv_sb = []
            hid_sb = [[None, None] for _ in range(NEC)]
            for e in range(NEC):
                wq_tile = wts_pool.tile([P, 512], bt, name=f"wqk_{e}", tag="wqk",
                                        bufs=NEC)
                nc.gpsimd.dma_start(wq_tile[:], wqkT[e * P:(e + 1) * P, :])
                wqk_sb.append(wq_tile)
                ht = hid_pool.tile([P, 1024], bt, name=f"hid_{e}_0",
                                   tag="hid", bufs=2 * NEC)
                heng = nc.sync if e % 2 == 0 else nc.scalar
                heng.dma_start(ht[:], hiddenT[e * P:(e + 1) * P, 0:1024])
                hid_sb[e][0] = ht
            for e in range(NEC):
                wv_tile = wts_pool.tile([P, 256], bt, name=f"wv_{e}", tag="wv",
                                        bufs=NEC)
                nc.gpsimd.dma_start(wv_tile[:], wvT[e * P:(e + 1) * P, :])
                wv_sb.append(wv_tile)
            nc.gpsimd.dma_start(cos_sb[:], cosT)
            nc.gpsimd.dma_start(sin_sb[:], sinT)
            nc.gpsimd.dma_start(mask_sb[:], masksT)
            for e in range(NEC):
                ht = hid_pool.tile([P, 1024], bt, name=f"hid_{e}_1",
                                   tag="hid", bufs=2 * NEC)
                heng = nc.sync if e % 2 == 0 else nc.scalar
                heng.dma_start(ht[:], hiddenT[e * P:(e + 1) * P, 1024:2048])
                hid_sb[e][1] = ht
            nc.gpsimd.dma_start(wo_sb[:], woutT.rearrange("p (o e) -> p o e",
                                                          o=NEC))

            oti_sb = []
            oti3_sb = []

            def hidsl(e, lo, width):  # [lo, lo+width) columns of chunk e
                half = hid_sb[e][lo // 1024]
                off = lo % 1024
                return half[:, off:off + width]

            def do_oproj(jj):
                qs = slice(512 * jj, 512 * (jj + 1))
                pps = [pp_pool.tile([P, 512], f32, name=f"pps{jj}_{b}",
                                    tag="gps") for b in range(2)]
                for fc in range(NEC):
                    for b in range(2):
                        nc.tensor.matmul(
                            pps[b][:], wo_sb[:, fc, b * P:(b + 1) * P],
                            oti_sb[jj][:, fc, :], start=(fc == 0),
                            stop=(fc == NEC - 1))
                for b in range(2):
                    ob = op_pool.tile([P, 512], f32, name=f"ob{jj}_{b}",
                                      tag="ob")
                    nc.scalar.copy(ob[:], pps[b][:])
                    nc.sync.dma_start(outT[b * P:(b + 1) * P, qs], ob[:])

            def do_oproj3():
                # chunk 3: even fc rows come from the head-0 AG, odd fc rows
                # from the head-1 AG; accumulate both phases into one PSUM
                # chain so the even half can run while head-1's AG is in
                # flight.
                qs = slice(1536, 2048)
                pps = [pp_pool.tile([P, 512], f32, name=f"pps3_{b}", tag="gps")
                       for b in range(2)]
                for phase in range(2):
                    for k in range(8):
                        fc = 2 * k + phase
                        for b in range(2):
                            nc.tensor.matmul(
                                pps[b][:], wo_sb[:, fc, b * P:(b + 1) * P],
                                oti3_sb[phase][:, k, :],
                                start=(phase == 0 and k == 0),
                                stop=(phase == 1 and k == 7))
                for b in range(2):
                    ob = op_pool.tile([P, 512], f32, name=f"ob3_{b}", tag="ob")
                    nc.scalar.copy(ob[:], pps[b][:])
                    nc.sync.dma_start(outT[b * P:(b + 1) * P, qs], ob[:])

            def do_qkv(j):
                qs = slice(512 * j, 512 * (j + 1))
                # ---- qkv chunk j: rot groups in 2 waves of 2 psum banks ----
                g01 = []
                for g in (0, 1):
                    gp = gps_pool.tile([P, 512], f32, name=f"gps{j}_{g}",
                                       tag="gps")
                    for e in range(NEC):
                        nc.tensor.matmul(
                            gp[:], wqk_sb[e][:, g * P:(g + 1) * P],
                            hidsl(e, 512 * j, 512), start=(e == 0),
                            stop=(e == NEC - 1))
                    gc = g01_pool.tile([P, 512], bt, name=f"g01_{j}_{g}",
                                       tag=f"g01_{g}")
                    nc.scalar.copy(gc[:], gp[:])
                    g01.append(gc)
                for g in (2, 3):
                    gp = gps_pool.tile([P, 512], f32, name=f"gps{j}_{g}",
                                       tag="gps")
                    for e in range(NEC):
                        nc.tensor.matmul(
                            gp[:], wqk_sb[e][:, g * P:(g + 1) * P],
                            hidsl(e, 512 * j, 512), start=(e == 0),
                            stop=(e == NEC - 1))
                    dst = qc if g == 2 else kc
                    nc.vector.tensor_copy(dst[0][j][64:128, :], gp[0:64, :])
                    nc.vector.tensor_copy(dst[1][j][64:128, :], gp[64:128, :])
                # v chunk j
                for st in range(4):
                    vp = gps_pool.tile([P, 256], f32, name=f"vps{j}_{st}",
                                       tag="gps")
                    for e in range(NEC):
                        nc.tensor.matmul(
                            vp[:], hidsl(e, 512 * j + st * P, P),
                            wv_sb[e][:], start=(e == 0), stop=(e == NEC - 1))
                    nc.vector.tensor_copy(vc[0][j][:, st * P:(st + 1) * P],
                                          vp[:, 0:P])
                    nc.vector.tensor_copy(vc[1][j][:, st * P:(st + 1) * P],
                                          vp[:, P:2 * P])
                # rope chunk j
                t0 = rtmp_pool.tile([P, 512], bt, name=f"t0_{j}", tag="t0")
                t1 = rtmp_pool.tile([P, 512], bt, name=f"t1_{j}", tag="t1")
                ta = rtmp_pool.tile([P, 512], bt, name=f"ta_{j}", tag="ta")
                tb = rtmp_pool.tile([P, 512], bt, name=f"tb_{j}", tag="tb")
                nc.vector.tensor_mul(t0[:], g01[0][:], cos_sb[:, qs])
                nc.vector.tensor_mul(t1[:], g01[1][:], sin_sb[:, qs])
                nc.vector.tensor_sub(ta[:], t0[:], t1[:])      # tops
                nc.vector.tensor_mul(t0[:], g01[1][:], cos_sb[:, qs])
                nc.vector.tensor_mul(t1[:], g01[0][:], sin_sb[:, qs])
                nc.vector.tensor_add(tb[:], t0[:], t1[:])      # bottoms
                for pc, dst in enumerate((qc[0], qc[1], kc[0], kc[1])):
                    ps = slice(32 * pc, 32 * (pc + 1))
                    nc.vector.tensor_copy(dst[j][0:32, :], ta[ps, :])
                    nc.vector.tensor_copy(dst[j][32:64, :], tb[ps, :])

            def do_attn(j, hi):
                nk = 4 * j + 4
                otp = ot_pool.tile([P, 512], f32, name=f"otp{hi}_{j}",
                                   tag="otp")
                dnp = dn_pool.tile([P, 512], f32, name=f"dnp{hi}_{j}",
                                   tag="dnp")
                for i in range(nk):
                    scp = sc_pool.tile([P, 512], f32, name=f"scp{hi}{j}{i}",
                                       tag="scp")
                    nc.tensor.matmul(
                        scp[:], kc[hi][i // 4][:, (i % 4) * P:
                                               (i % 4 + 1) * P],
                        qc[hi][j][:], start=True, stop=True)
                    pt = pt_pool.tile([P, 512], bt, name=f"pt{hi}{j}{i}",
                                      tag="pt")
                    nc.scalar.activation(
                        pt[:], scp[:], mybir.ActivationFunctionType.Exp,
                        scale=SCALE)
                    if i >= 4 * j:
                        off = 384 - 128 * (i - 4 * j)
                        nc.vector.tensor_mul(
                            pt[:], pt[:], mask_sb[:, off:off + 512])
                    nc.tensor.matmul(
                        otp[:], vc[hi][i // 4][:, (i % 4) * P:
                                               (i % 4 + 1) * P], pt[:],
                        start=(i == 0), stop=(i == nk - 1))
                    nc.tensor.matmul(
                        dnp[:], ones_sb[:], pt[:],
                        start=(i == 0), stop=(i == nk - 1))
                den_sb = oo_pool.tile([P, 512], f32, name=f"den{hi}_{j}",
                                      tag="den")
                nc.vector.reciprocal_approx_fast(den_sb[:], dnp[:])
                otn = oo_pool.tile([P, 512], bt, name=f"otn{hi}_{j}",
                                   tag="otn")
                nc.vector.tensor_mul(otn[:], otp[:], den_sb[:])
                if j < NQ - 1:
                    nc.sync.dma_start(ag_in[j][hi * P:(hi + 1) * P, :],
                                      otn[:])
                else:
                    nc.sync.dma_start(ag_in3[hi][:], otn[:])

            def fire_ag(j):
                nc.gpsimd.collective_compute(
                    "AllGather",
                    mybir.AluOpType.bypass,
                    replica_groups=[list(range(NCORES))],
                    ins=[ag_in[j].opt()],
                    outs=[ag_out[j].opt()],
                )
                # prefetch the gathered O^T as one contiguous 2MB load
                oti = oi_pool.tile([P, NEC, 512], bt, name=f"oti{j}",
                                   tag="oti")
                nc.gpsimd.dma_start(
                    oti[:], ag_out[j].rearrange("(o p) s -> p o s", p=P))
                oti_sb.append(oti)

            def fire_ag3(hi):
                nc.gpsimd.collective_compute(
                    "AllGather",
                    mybir.AluOpType.bypass,
                    replica_groups=[list(range(NCORES))],
                    ins=[ag_in3[hi].opt()],
                    outs=[ag_out3[hi].opt()],
                )
                oti3 = oi3_pool.tile([P, 8, 512], bt, name=f"oti3{hi}",
                                     tag=f"oti3{hi}")
                nc.gpsimd.dma_start(
                    oti3[:], ag_out3[hi].rearrange("(o p) s -> p o s", p=P))
                oti3_sb.append(oti3)

            # schedule: chunks 0/1 pipelined; chunk 3 head 0's AG fires while
            # chunk 2's attention runs, chunk 2's AG while chunk 3 head 1
            # runs, so the CC engine stays busy and the final AG is the small
            # per-head one; all output projections run at the end, filling
            # the last AG's latency with PE work.
            do_qkv(0)
            do_attn(0, 0)
            do_attn(0, 1)
            fire_ag(0)
            do_qkv(1)
            do_attn(1, 0)
            do_attn(1, 1)
            fire_ag(1)
            do_qkv(2)
            do_qkv(3)
            do_attn(3, 0)
            fire_ag3(0)
            do_attn(2, 0)
            do_attn(2, 1)
            fire_ag(2)
            do_attn(3, 1)
            fire_ag3(1)
            do_oproj(0)
            do_oproj(1)
            do_oproj(2)
            do_oproj3()


def _build_program():
    import concourse.bass as bass  # noqa: F401
    import concourse.mybir as mybir
    import concourse.tile as tile
    from concourse import bacc

    nc = bacc.Bacc("TRN2", target_bir_lowering=False, debug=False,
                   enable_asserts=False, num_devices=NCORES)
    f32 = mybir.dt.float32
    bt = mybir.dt.bfloat16
    hiddenT = nc.dram_tensor("hiddenT", [E, S], bt, kind="ExternalInput").ap()
    wqkT = nc.dram_tensor("wqkT", [E, 512], bt, kind="ExternalInput").ap()
    wvT = nc.dram_tensor("wvT", [E, 256], bt, kind="ExternalInput").ap()
    woutT = nc.dram_tensor("woutT", [P, NEC * 256], bt, kind="ExternalInput").ap()
    cosT = nc.dram_tensor("cosT", [P, S], bt, kind="ExternalInput").ap()
    sinT = nc.dram_tensor("sinT", [P, S], bt, kind="ExternalInput").ap()
    masks = nc.dram_tensor("masks", [P, 896], bt, kind="ExternalInput").ap()
    outT = nc.dram_tensor("outT", [2 * P, S], f32, kind="ExternalOutput").ap()

    with tile.TileContext(nc) as tc:
        _kernel_body(tc, outT, hiddenT, wqkT, wvT, woutT, cosT, sinT, masks)
    nc.compile()
    return nc


def get_program():
    if "nc" not in _CACHE:
        _CACHE["nc"] = _build_program()
    return _CACHE["nc"]


def _install_ntff_shim():
    """Provide antenv.axon_hooks (missing in this image) so trace=True works."""
    import sys
    import types
    try:
        import antenv.axon_hooks  # noqa: F401
        return
    except ImportError:
        pass
    import antenv
    mod = types.ModuleType("antenv.axon_hooks")
    mod._hook = None

    def set_axon_ntff_profile_hook(h):
        mod._hook = h

    def get_axon_ntff_profile_hook():
        return mod._hook

    mod.set_axon_ntff_profile_hook = set_axon_ntff_profile_hook
    mod.get_axon_ntff_profile_hook = get_axon_ntff_profile_hook
    sys.modules["antenv.axon_hooks"] = mod
    antenv.axon_hooks = mod
    try:
        from trn_agent_boot.trn_boot import _ntff_profile_via_ctypes
        hook = _ntff_profile_via_ctypes("/opt/axon/libaxon_pjrt.so")
        if hook is not None:
            mod._hook = hook
    except Exception:
        pass


def run(inputs, trace=False):
    """Run on the 8 NeuronCores; returns (out [1,S,E], BassKernelResults)."""
    from concourse import bass_utils

    if trace:
        _install_ntff_shim()
    nc = get_program()
    in_maps = _build_in_maps(inputs["hidden_states"], inputs["w_qkv"],
                             inputs["w_out"])
    res = bass_utils.run_bass_kernel_spmd(
        nc, in_maps, core_ids=list(range(NCORES)), trace=trace)
    outT = np.concatenate([res.results[c]["outT"] for c in range(NCORES)],
                          axis=0)  # [E, S]
    out = np.ascontiguousarray(outT.T).reshape(1, S, E).astype(np.float32)
    return out, res


def kernel(hidden_states, w_qkv, w_out):
    out, _ = run({"hidden_states": hidden_states, "w_qkv": w_qkv,
                  "w_out": w_out})
    return out


# revision 22
# speedup vs baseline: 1.0267x; 1.0130x over previous
"""CodeGen attention block (B=1, S=2048, E=2048, H=16, D=128, rot=64) on 8 TRN2
NeuronCores.

Sharding: tensor-parallel over heads (2 heads/core). Each core computes its
heads' q/k in transposed [d, s] layout (partial rotary applied via a host-side
even/odd channel permutation folded into the qkv weights, which turns the
interleaved rotation into a rotate-half form on contiguous partition blocks),
v in [s, d] layout, causal softmax attention entirely on-chip (scores
transposed [k, q] so the P·V matmul needs no transposes; softmax denominators
via a ones-stationary matmul), then AllGathers the per-core attention output
O^T [256, 2048] and computes a 256-column slice of the output projection.
Host assembles the 8 slices and transposes.

All PE-facing tensors are bf16 (full PE speed, half the DMA); accumulation
is fp32 in PSUM. The AllGather is split per-s-chunk (and per-head for the
last chunk) so collectives fire as soon as the producing attention work
finishes, overlapping comm with attention; the output projection for chunk j
is interleaved into the main loop after chunk j+2's attention so PE never
waits on a collective except at the very tail.
"""

import numpy as np

H, D, ROT, MP = 16, 128, 64, 4
S, E = 2048, 2048
NCORES = 8
P = 128
NQ = 4            # 512-wide q chunks
NKT = S // P      # 16 k tiles
NEC = E // P      # 16 contraction chunks
SCALE = float(1.0 / np.sqrt(np.float64(D)))

_CACHE = {}


# ----------------------------------------------------------------------------
# host-side input prep
# ----------------------------------------------------------------------------

def _head_rows(h):
    g, j = h // 4, h % 4
    base = g * (3 * 512)
    q = np.arange(base + j * 128, base + (j + 1) * 128)
    v = np.arange(base + 512 + j * 128, base + 512 + (j + 1) * 128)
    k = np.arange(base + 1024 + j * 128, base + 1024 + (j + 1) * 128)
    return q, v, k


def _prep_core_weights(c, w_qkv, w_out):
    h0, h1 = 2 * c, 2 * c + 1
    top = np.arange(0, ROT, 2)
    bot = np.arange(1, ROT, 2)
    wq, wk, wv = {}, {}, {}
    for h in (h0, h1):
        qr, vr, kr = _head_rows(h)
        wq[h], wv[h], wk[h] = w_qkv[qr], w_qkv[vr], w_qkv[kr]
    G0 = np.concatenate([wq[h0][top], wq[h1][top], wk[h0][top], wk[h1][top]], 0)
    G1 = np.concatenate([wq[h0][bot], wq[h1][bot], wk[h0][bot], wk[h1][bot]], 0)
    G2 = np.concatenate([wq[h0][ROT:], wq[h1][ROT:]], 0)
    G3 = np.concatenate([wk[h0][ROT:], wk[h1][ROT:]], 0)
    wqkT = np.ascontiguousarray(
        np.concatenate([G0, G1, G2, G3], 0).T, dtype=np.float32)       # [E, 512]
    wvT = np.ascontiguousarray(
        np.concatenate([wv[h0], wv[h1]], 0).T, dtype=np.float32)       # [E, 256]
    woutT = np.ascontiguousarray(
        w_out[256 * c:256 * (c + 1), :].T, dtype=np.float32)           # [E, 256]
    # pre-pack for a fully contiguous [128, 16, 256] SBUF load
    woutT = np.ascontiguousarray(
        woutT.reshape(16, 128, 256).transpose(1, 0, 2)).reshape(128, 16 * 256)
    return wqkT, wvT, woutT


def _cos_sin():
    inv_freq = 1.0 / (10000.0 ** (np.arange(0, ROT, 2, dtype=np.float32) / ROT))
    ang = np.arange(S, dtype=np.float32)[:, None] * inv_freq[None, :]
    cosb = np.cos(ang).T.astype(np.float32)        # [32, S]
    sinb = np.sin(ang).T.astype(np.float32)
    return (np.ascontiguousarray(np.tile(cosb, (4, 1))),
            np.ascontiguousarray(np.tile(sinb, (4, 1))))               # [128, S]


def _mask_tiles():
    # single shifted mask: M[p, x] = (p <= x - 384); the diagonal-offset-mm
    # mask is the 512-wide view starting at column 384 - 128*mm
    pp = np.arange(128)[:, None]
    xx = np.arange(896)[None, :]
    return (xx >= pp + 384).astype(np.float32)


def _build_in_maps(hidden_states, w_qkv, w_out):
    import ml_dtypes
    bf16 = ml_dtypes.bfloat16
    hiddenT = np.ascontiguousarray(
        np.asarray(hidden_states, np.float32).reshape(S, E).T).astype(bf16)
    COS, SIN = _cos_sin()
    COS, SIN = COS.astype(bf16), SIN.astype(bf16)
    masks = _mask_tiles().astype(bf16)
    in_maps = []
    for c in range(NCORES):
        wqkT, wvT, woutT = _prep_core_weights(
            c, np.asarray(w_qkv, np.float32), np.asarray(w_out, np.float32))
        in_maps.append({
            "hiddenT": hiddenT,
            "wqkT": wqkT.astype(bf16),
            "wvT": wvT.astype(bf16),
            "woutT": woutT.astype(bf16),
            "cosT": COS,
            "sinT": SIN,
            "masks": masks,
        })
    return in_maps


# ----------------------------------------------------------------------------
# device program
# ----------------------------------------------------------------------------

def _kernel_body(tc, outT, hiddenT, wqkT, wvT, woutT, cosT, sinT, masksT):
    import concourse.mybir as mybir
    from contextlib import ExitStack

    nc = tc.nc
    f32 = mybir.dt.float32
    bt = mybir.dt.bfloat16

    with ExitStack() as ctx:
        const = ctx.enter_context(tc.tile_pool(name="const", bufs=1))
        mask_sb = const.tile([P, 896], bt, name="mask_sb")
        ones_sb = const.tile([P, P], bt, name="ones_sb")
        wo_sb = const.tile([P, NEC, 256], bt, name="wo_sb")
        nc.vector.memset(ones_sb[:], 1.0)

        dram = ctx.enter_context(tc.tile_pool(name="dram", bufs=1, space="DRAM"))
        ag_in = [dram.tile([2 * P, 512], bt, name=f"ag_in{j}", tag=f"ag_in{j}")
                 for j in range(NQ - 1)]
        ag_out = [dram.tile([E, 512], bt, name=f"ag_out{j}", tag=f"ag_out{j}",
                            addr_space="Shared") for j in range(NQ - 1)]
        # last chunk: per-head collectives so comm starts as soon as head 0
        # finishes and the tail only waits on head 1's small AG
        ag_in3 = [dram.tile([P, 512], bt, name=f"ag_in3{h}", tag=f"ag_in3{h}")
                  for h in range(2)]
        ag_out3 = [dram.tile([8 * P, 512], bt, name=f"ag_out3{h}",
                             tag=f"ag_out3{h}", addr_space="Shared")
                   for h in range(2)]

        # --- phase 1+2 interleaved: per-chunk qkv -> attention -> AG --------
        with tc.tile_pool(name="ph1c", bufs=1) as ph1c_pool, \
             tc.tile_pool(name="chunk", bufs=1) as ck_pool, \
             tc.tile_pool(name="hid", bufs=2 * NEC) as hid_pool, \
             tc.tile_pool(name="wts", bufs=NEC) as wts_pool, \
             tc.tile_pool(name="g01c", bufs=1) as g01_pool, \
             tc.tile_pool(name="rtmp", bufs=1) as rtmp_pool, \
             tc.tile_pool(name="pt", bufs=5) as pt_pool, \
             tc.tile_pool(name="pts", bufs=2) as pts_pool, \
             tc.tile_pool(name="oout", bufs=2) as oo_pool, \
             tc.tile_pool(name="otin", bufs=2) as oi_pool, \
             tc.tile_pool(name="otin3", bufs=1) as oi3_pool, \
             tc.tile_pool(name="oprj", bufs=2) as op_pool, \
             tc.tile_pool(name="gps", bufs=2, space="PSUM") as gps_pool, \
             tc.tile_pool(name="scps", bufs=2, space="PSUM") as sc_pool, \
             tc.tile_pool(name="otps", bufs=2, space="PSUM") as ot_pool, \
             tc.tile_pool(name="dnps", bufs=2, space="PSUM") as dn_pool:

            pp_pool = gps_pool  # oproj PSUM reuses the (idle-by-then) qkv banks

            cos_sb = ph1c_pool.tile([P, S], bt, name="cos_sb", tag="cos_sb")
            sin_sb = ph1c_pool.tile([P, S], bt, name="sin_sb", tag="sin_sb")

            # per-chunk activations: q/k transposed [d, 512]; v [k-in-tile, 4*128]
            # q is only live from its qkv until its attention -> cycle 2 bufs
            qc = [[ck_pool.tile([P, 512], bt, name=f"qc{h}_{j}", tag=f"qc{h}",
                                bufs=2)
                   for j in range(NQ)] for h in range(2)]
            kc = [[ck_pool.tile([P, 512], bt, name=f"kc{h}_{j}", tag=f"kc{h}_{j}")
                   for j in range(NQ)] for h in range(2)]
            vc = [[ck_pool.tile([P, 512], bt, name=f"vc{h}_{j}", tag=f"vc{h}_{j}")
                   for j in range(NQ)] for h in range(2)]

            # ---- input loads, priority-ordered per DMA ring ----------------
            # critical first: wqk (gpsimd ring) + hidden half-0 (sync/scalar/
            # vector rings round-robin), then wv, cos/sin/masks, hidden
            # half-1, and wo last (only needed by the output projection).
            wqk_sb = []
            wv_sb = []
            hid_sb = [[None, None] for _ in range(NEC)]
            for e in range(NEC):
                wq_tile = wts_pool.tile([P, 512], bt, name=f"wqk_{e}", tag="wqk",
                                        bufs=NEC)
                nc.gpsimd.dma_start(wq_tile[:], wqkT[e * P:(e + 1) * P, :])
                wqk_sb.append(wq_tile)
                ht = hid_pool.tile([P, 1024], bt, name=f"hid_{e}_0",
                                   tag="hid", bufs=2 * NEC)
                heng = nc.sync if e % 2 == 0 else nc.scalar
                heng.dma_start(ht[:], hiddenT[e * P:(e + 1) * P, 0:1024])
                hid_sb[e][0] = ht
            for e in range(NEC):
                wv_tile = wts_pool.tile([P, 256], bt, name=f"wv_{e}", tag="wv",
                                        bufs=NEC)
                nc.gpsimd.dma_start(wv_tile[:], wvT[e * P:(e + 1) * P, :])
                wv_sb.append(wv_tile)
            nc.gpsimd.dma_start(cos_sb[:], cosT)
            nc.gpsimd.dma_start(sin_sb[:], sinT)
            nc.gpsimd.dma_start(mask_sb[:], masksT)
            for e in range(NEC):
                ht = hid_pool.tile([P, 1024], bt, name=f"hid_{e}_1",
                                   tag="hid", bufs=2 * NEC)
                heng = nc.sync if e % 2 == 0 else nc.scalar
                heng.dma_start(ht[:], hiddenT[e * P:(e + 1) * P, 1024:2048])
                hid_sb[e][1] = ht
            nc.gpsimd.dma_start(wo_sb[:], woutT.rearrange("p (o e) -> p o e",
                                                          o=NEC))

            oti_sb = []
            oti3_sb = []

            def hidsl(e, lo, width):  # [lo, lo+width) columns of chunk e
                half = hid_sb[e][lo // 1024]
                off = lo % 1024
                return half[:, off:off + width]

            def do_oproj(jj):
                qs = slice(512 * jj, 512 * (jj + 1))
                pps = [pp_pool.tile([P, 512], f32, name=f"pps{jj}_{b}",
                                    tag="gps") for b in range(2)]
                for fc in range(NEC):
                    for b in range(2):
                        nc.tensor.matmul(
                            pps[b][:], wo_sb[:, fc, b * P:(b + 1) * P],
                            oti_sb[jj][:, fc, :], start=(fc == 0),
                            stop=(fc == NEC - 1))
                for b in range(2):
                    ob = op_pool.tile([P, 512], bt, name=f"ob{jj}_{b}",
                                      tag="ob")
                    nc.scalar.copy(ob[:], pps[b][:])
                    nc.sync.dma_start(outT[b * P:(b + 1) * P, qs], ob[:])

            def do_oproj3():
                # chunk 3: even fc rows come from the head-0 AG, odd fc rows
                # from the head-1 AG; accumulate both phases into one PSUM
                # chain so the even half can run while head-1's AG is in
                # flight.
                qs = slice(1536, 2048)
                pps = [pp_pool.tile([P, 512], f32, name=f"pps3_{b}", tag="gps")
                       for b in range(2)]
                for phase in range(2):
                    for k in range(8):
                        fc = 2 * k + phase
                        for b in range(2):
                            nc.tensor.matmul(
                                pps[b][:], wo_sb[:, fc, b * P:(b + 1) * P],
                                oti3_sb[phase][:, k, :],
                                start=(phase == 0 and k == 0),
                                stop=(phase == 1 and k == 7))
                for b in range(2):
                    ob = op_pool.tile([P, 512], bt, name=f"ob3_{b}", tag="ob")
                    nc.scalar.copy(ob[:], pps[b][:])
                    nc.sync.dma_start(outT[b * P:(b + 1) * P, qs], ob[:])

            def do_qkv(j):
                qs = slice(512 * j, 512 * (j + 1))
                # ---- qkv chunk j: rot groups in 2 waves of 2 psum banks ----
                g01 = []
                for g in (0, 1):
                    gp = gps_pool.tile([P, 512], f32, name=f"gps{j}_{g}",
                                       tag="gps")
                    for e in range(NEC):
                        nc.tensor.matmul(
                            gp[:], wqk_sb[e][:, g * P:(g + 1) * P],
                            hidsl(e, 512 * j, 512), start=(e == 0),
                            stop=(e == NEC - 1))
                    gc = g01_pool.tile([P, 512], bt, name=f"g01_{j}_{g}",
                                       tag=f"g01_{g}")
                    nc.scalar.copy(gc[:], gp[:])
                    g01.append(gc)
                for g in (2, 3):
                    gp = gps_pool.tile([P, 512], f32, name=f"gps{j}_{g}",
                                       tag="gps")
                    for e in range(NEC):
                        nc.tensor.matmul(
                            gp[:], wqk_sb[e][:, g * P:(g + 1) * P],
                            hidsl(e, 512 * j, 512), start=(e == 0),
                            stop=(e == NEC - 1))
                    dst = qc if g == 2 else kc
                    nc.vector.tensor_copy(dst[0][j][64:128, :], gp[0:64, :])
                    nc.vector.tensor_copy(dst[1][j][64:128, :], gp[64:128, :])
                # v chunk j
                for st in range(4):
                    vp = gps_pool.tile([P, 256], f32, name=f"vps{j}_{st}",
                                       tag="gps")
                    for e in range(NEC):
                        nc.tensor.matmul(
                            vp[:], hidsl(e, 512 * j + st * P, P),
                            wv_sb[e][:], start=(e == 0), stop=(e == NEC - 1))
                    nc.vector.tensor_copy(vc[0][j][:, st * P:(st + 1) * P],
                                          vp[:, 0:P])
                    nc.vector.tensor_copy(vc[1][j][:, st * P:(st + 1) * P],
                                          vp[:, P:2 * P])
                # rope chunk j
                t0 = rtmp_pool.tile([P, 512], bt, name=f"t0_{j}", tag="t0")
                t1 = rtmp_pool.tile([P, 512], bt, name=f"t1_{j}", tag="t1")
                ta = rtmp_pool.tile([P, 512], bt, name=f"ta_{j}", tag="ta")
                tb = rtmp_pool.tile([P, 512], bt, name=f"tb_{j}", tag="tb")
                nc.vector.tensor_mul(t0[:], g01[0][:], cos_sb[:, qs])
                nc.vector.tensor_mul(t1[:], g01[1][:], sin_sb[:, qs])
                nc.vector.tensor_sub(ta[:], t0[:], t1[:])      # tops
                nc.vector.tensor_mul(t0[:], g01[1][:], cos_sb[:, qs])
                nc.vector.tensor_mul(t1[:], g01[0][:], sin_sb[:, qs])
                nc.vector.tensor_add(tb[:], t0[:], t1[:])      # bottoms
                for pc, dst in enumerate((qc[0], qc[1], kc[0], kc[1])):
                    ps = slice(32 * pc, 32 * (pc + 1))
                    nc.vector.tensor_copy(dst[j][0:32, :], ta[ps, :])
                    nc.vector.tensor_copy(dst[j][32:64, :], tb[ps, :])

            def do_attn(j, hi):
                nk = 4 * j + 4
                otp = ot_pool.tile([P, 512], f32, name=f"otp{hi}_{j}",
                                   tag="otp")
                dnp = dn_pool.tile([P, 512], f32, name=f"dnp{hi}_{j}",
                                   tag="dnp")
                prev_pt = None
                for i in range(nk):
                    scp = sc_pool.tile([P, 512], f32, name=f"scp{hi}{j}{i}",
                                       tag="scp")
                    nc.tensor.matmul(
                        scp[:], kc[hi][i // 4][:, (i % 4) * P:
                                               (i % 4 + 1) * P],
                        qc[hi][j][:], start=True, stop=True)
                    pt = pt_pool.tile([P, 512], bt, name=f"pt{hi}{j}{i}",
                                      tag="pt")
                    nc.scalar.activation(
                        pt[:], scp[:], mybir.ActivationFunctionType.Exp,
                        scale=SCALE)
                    if i >= 4 * j:
                        off = 384 - 128 * (i - 4 * j)
                        nc.vector.tensor_mul(
                            pt[:], pt[:], mask_sb[:, off:off + 512])
                    nc.tensor.matmul(
                        otp[:], vc[hi][i // 4][:, (i % 4) * P:
                                               (i % 4 + 1) * P], pt[:],
                        start=(i == 0), stop=(i == nk - 1))
                    # denominator: pre-add pt pairs on the DVE (bf16, cheap)
                    # so only nk/2 ones-matmuls hit the PE
                    if i % 2 == 1:
                        ps2 = pts_pool.tile([P, 512], bt,
                                            name=f"ps2{hi}{j}{i}", tag="ps2")
                        nc.vector.tensor_add(ps2[:], prev_pt[:], pt[:])
                        nc.tensor.matmul(
                            dnp[:], ones_sb[:], ps2[:],
                            start=(i == 1), stop=(i == nk - 1))
                    prev_pt = pt
                den_sb = oo_pool.tile([P, 512], f32, name=f"den{hi}_{j}",
                                      tag="den")
                nc.vector.reciprocal_approx_fast(den_sb[:], dnp[:])
                otn = oo_pool.tile([P, 512], bt, name=f"otn{hi}_{j}",
                                   tag="otn")
                nc.vector.tensor_mul(otn[:], otp[:], den_sb[:])
                if j < NQ - 1:
                    nc.sync.dma_start(ag_in[j][hi * P:(hi + 1) * P, :],
                                      otn[:])
                else:
                    nc.sync.dma_start(ag_in3[hi][:], otn[:])

            def prefetch_oti(j, pieces=1):
                # stage gathered O^T; pieces let the consumer start on piece
                # 0 while later pieces stream
                oti = oi_pool.tile([P, NEC, 512], bt, name=f"oti{j}",
                                   tag="oti")
                w = NEC // pieces
                for k in range(pieces):
                    nc.gpsimd.dma_start(
                        oti[:, w * k:w * (k + 1), :],
                        ag_out[j][P * w * k:P * w * (k + 1), :].rearrange(
                            "(o p) s -> p o s", p=P))
                oti_sb.append(oti)

            def prefetch_oti3(hi, pieces=1):
                oti3 = oi3_pool.tile([P, 8, 512], bt, name=f"oti3{hi}",
                                     tag=f"oti3{hi}")
                w = 8 // pieces
                for k in range(pieces):
                    nc.gpsimd.dma_start(
                        oti3[:, w * k:w * (k + 1), :],
                        ag_out3[hi][P * w * k:P * w * (k + 1), :].rearrange(
                            "(o p) s -> p o s", p=P))
                oti3_sb.append(oti3)

            def fire_ag(j, prefetch=True):
                nc.gpsimd.collective_compute(
                    "AllGather",
                    mybir.AluOpType.bypass,
                    replica_groups=[list(range(NCORES))],
                    ins=[ag_in[j].opt()],
                    outs=[ag_out[j].opt()],
                )
                if prefetch:
                    prefetch_oti(j)

            def fire_ag3(hi, prefetch=True):
                nc.gpsimd.collective_compute(
                    "AllGather",
                    mybir.AluOpType.bypass,
                    replica_groups=[list(range(NCORES))],
                    ins=[ag_in3[hi].opt()],
                    outs=[ag_out3[hi].opt()],
                )
                if prefetch:
                    prefetch_oti3(hi)

            # schedule: chunks 0/1 pipelined; chunk 3 head 0's AG fires while
            # chunk 2's attention runs, chunk 2's AG while chunk 3 head 1
            # runs, so the CC engine stays busy and the final AG is the small
            # per-head one; all output projections run at the end, filling
            # the last AG's latency with PE work.
            do_qkv(0)
            do_attn(0, 0)
            do_attn(0, 1)
            fire_ag(0)
            do_qkv(1)
            do_attn(1, 0)
            do_attn(1, 1)
            fire_ag(1)
            do_qkv(2)
            do_qkv(3)
            do_attn(3, 0)
            fire_ag3(0)
            do_attn(2, 0)
            do_attn(2, 1)
            fire_ag(2, prefetch=False)
            do_attn(3, 1)
            fire_ag3(1, prefetch=False)
            # prefetches after both AG issues: a DIRECT2D blocked on its
            # AG-done semaphore must never delay an AG issue behind it
            prefetch_oti(2, pieces=4)
            prefetch_oti3(1, pieces=2)
            do_oproj(0)
            do_oproj(1)
            do_oproj(2)
            do_oproj3()


def _build_program():
    import concourse.bass as bass  # noqa: F401
    import concourse.mybir as mybir
    import concourse.tile as tile
    from concourse import bacc

    nc = bacc.Bacc("TRN2", target_bir_lowering=False, debug=False,
                   enable_asserts=False, num_devices=NCORES)
    f32 = mybir.dt.float32
    bt = mybir.dt.bfloat16
    hiddenT = nc.dram_tensor("hiddenT", [E, S], bt, kind="ExternalInput").ap()
    wqkT = nc.dram_tensor("wqkT", [E, 512], bt, kind="ExternalInput").ap()
    wvT = nc.dram_tensor("wvT", [E, 256], bt, kind="ExternalInput").ap()
    woutT = nc.dram_tensor("woutT", [P, NEC * 256], bt, kind="ExternalInput").ap()
    cosT = nc.dram_tensor("cosT", [P, S], bt, kind="ExternalInput").ap()
    sinT = nc.dram_tensor("sinT", [P, S], bt, kind="ExternalInput").ap()
    masks = nc.dram_tensor("masks", [P, 896], bt, kind="ExternalInput").ap()
    outT = nc.dram_tensor("outT", [2 * P, S], bt, kind="ExternalOutput").ap()

    with tile.TileContext(nc) as tc:
        _kernel_body(tc, outT, hiddenT, wqkT, wvT, woutT, cosT, sinT, masks)
    nc.compile()
    return nc


def get_program():
    if "nc" not in _CACHE:
        _CACHE["nc"] = _build_program()
    return _CACHE["nc"]


def _install_ntff_shim():
    """Provide antenv.axon_hooks (missing in this image) so trace=True works."""
    import sys
    import types
    try:
        import antenv.axon_hooks  # noqa: F401
        return
    except ImportError:
        pass
    import antenv
    mod = types.ModuleType("antenv.axon_hooks")
    mod._hook = None

    def set_axon_ntff_profile_hook(h):
        mod._hook = h

    def get_axon_ntff_profile_hook():
        return mod._hook

    mod.set_axon_ntff_profile_hook = set_axon_ntff_profile_hook
    mod.get_axon_ntff_profile_hook = get_axon_ntff_profile_hook
    sys.modules["antenv.axon_hooks"] = mod
    antenv.axon_hooks = mod
    try:
        from trn_agent_boot.trn_boot import _ntff_profile_via_ctypes
        hook = _ntff_profile_via_ctypes("/opt/axon/libaxon_pjrt.so")
        if hook is not None:
            mod._hook = hook
    except Exception:
        pass


def run(inputs, trace=False):
    """Run on the 8 NeuronCores; returns (out [1,S,E], BassKernelResults)."""
    from concourse import bass_utils

    if trace:
        _install_ntff_shim()
    nc = get_program()
    in_maps = _build_in_maps(inputs["hidden_states"], inputs["w_qkv"],
                             inputs["w_out"])
    res = bass_utils.run_bass_kernel_spmd(
        nc, in_maps, core_ids=list(range(NCORES)), trace=trace)
    outT = np.concatenate(
        [np.asarray(res.results[c]["outT"], dtype=np.float32)
         for c in range(NCORES)], axis=0)  # [E, S]
    out = np.ascontiguousarray(outT.T).reshape(1, S, E).astype(np.float32)
    return out, res


def kernel(hidden_states, w_qkv, w_out):
    out, _ = run({"hidden_states": hidden_states, "w_qkv": w_qkv,
                  "w_out": w_out})
    return out


# revision 25
# speedup vs baseline: 1.0943x; 1.0658x over previous
"""CodeGen attention block (B=1, S=2048, E=2048, H=16, D=128, rot=64) on 8 TRN2
NeuronCores.

Sharding: tensor-parallel over heads (2 heads/core). Each core computes its
heads' q/k in transposed [d, s] layout (partial rotary applied via a host-side
even/odd channel permutation folded into the qkv weights, which turns the
interleaved rotation into a rotate-half form on contiguous partition blocks),
v in [s, d] layout, causal softmax attention entirely on-chip (scores
transposed [k, q] so the P·V matmul needs no transposes; softmax denominators
via a ones-stationary matmul), then AllGathers the per-core attention output
O^T [256, 2048] and computes a 256-column slice of the output projection.
Host assembles the 8 slices and transposes.

All PE-facing tensors are bf16 (full PE speed, half the DMA); accumulation
is fp32 in PSUM. The AllGather is split per-s-chunk (and per-head for the
last chunk) so collectives fire as soon as the producing attention work
finishes, overlapping comm with attention; the output projection for chunk j
is interleaved into the main loop after chunk j+2's attention so PE never
waits on a collective except at the very tail.
"""

import numpy as np

H, D, ROT, MP = 16, 128, 64, 4
S, E = 2048, 2048
NCORES = 8
P = 128
NQ = 4            # 512-wide q chunks
NKT = S // P      # 16 k tiles
NEC = E // P      # 16 contraction chunks
SCALE = float(1.0 / np.sqrt(np.float64(D)))

_CACHE = {}


# ----------------------------------------------------------------------------
# host-side input prep
# ----------------------------------------------------------------------------

def _head_rows(h):
    g, j = h // 4, h % 4
    base = g * (3 * 512)
    q = np.arange(base + j * 128, base + (j + 1) * 128)
    v = np.arange(base + 512 + j * 128, base + 512 + (j + 1) * 128)
    k = np.arange(base + 1024 + j * 128, base + 1024 + (j + 1) * 128)
    return q, v, k


def _prep_core_weights(c, w_qkv, w_out):
    h0, h1 = 2 * c, 2 * c + 1
    top = np.arange(0, ROT, 2)
    bot = np.arange(1, ROT, 2)
    wq, wk, wv = {}, {}, {}
    for h in (h0, h1):
        qr, vr, kr = _head_rows(h)
        wq[h], wv[h], wk[h] = w_qkv[qr], w_qkv[vr], w_qkv[kr]
    G0 = np.concatenate([wq[h0][top], wq[h1][top], wk[h0][top], wk[h1][top]], 0)
    G1 = np.concatenate([wq[h0][bot], wq[h1][bot], wk[h0][bot], wk[h1][bot]], 0)
    G2 = np.concatenate([wq[h0][ROT:], wq[h1][ROT:]], 0)
    G3 = np.concatenate([wk[h0][ROT:], wk[h1][ROT:]], 0)
    wqkT = np.ascontiguousarray(
        np.concatenate([G0, G1, G2, G3], 0).T, dtype=np.float32)       # [E, 512]
    wvT = np.ascontiguousarray(
        np.concatenate([wv[h0], wv[h1]], 0).T, dtype=np.float32)       # [E, 256]
    woutT = np.ascontiguousarray(
        w_out[256 * c:256 * (c + 1), :].T, dtype=np.float32)           # [E, 256]
    # pre-pack for a fully contiguous [128, 16, 256] SBUF load
    woutT = np.ascontiguousarray(
        woutT.reshape(16, 128, 256).transpose(1, 0, 2)).reshape(128, 16 * 256)
    return wqkT, wvT, woutT


def _cos_sin():
    inv_freq = 1.0 / (10000.0 ** (np.arange(0, ROT, 2, dtype=np.float32) / ROT))
    ang = np.arange(S, dtype=np.float32)[:, None] * inv_freq[None, :]
    cosb = np.cos(ang).T.astype(np.float32)        # [32, S]
    sinb = np.sin(ang).T.astype(np.float32)
    return (np.ascontiguousarray(np.tile(cosb, (4, 1))),
            np.ascontiguousarray(np.tile(sinb, (4, 1))))               # [128, S]


def _mask_tiles():
    # single shifted mask: M[p, x] = (p <= x - 384); the diagonal-offset-mm
    # mask is the 512-wide view starting at column 384 - 128*mm
    pp = np.arange(128)[:, None]
    xx = np.arange(896)[None, :]
    return (xx >= pp + 384).astype(np.float32)


def _build_in_maps(hidden_states, w_qkv, w_out):
    import ml_dtypes
    bf16 = ml_dtypes.bfloat16
    hiddenT = np.ascontiguousarray(
        np.asarray(hidden_states, np.float32).reshape(S, E).T).astype(bf16)
    COS, SIN = _cos_sin()
    COS, SIN = COS.astype(bf16), SIN.astype(bf16)
    masks = _mask_tiles().astype(bf16)
    in_maps = []
    for c in range(NCORES):
        wqkT, wvT, woutT = _prep_core_weights(
            c, np.asarray(w_qkv, np.float32), np.asarray(w_out, np.float32))
        in_maps.append({
            "hiddenT": hiddenT,
            "wqkT": wqkT.astype(bf16),
            "wvT": wvT.astype(bf16),
            "woutT": woutT.astype(bf16),
            "cosT": COS,
            "sinT": SIN,
            "masks": masks,
        })
    return in_maps


# ----------------------------------------------------------------------------
# device program
# ----------------------------------------------------------------------------

def _kernel_body(tc, outT, hiddenT, wqkT, wvT, woutT, cosT, sinT, masksT):
    import concourse.mybir as mybir
    from contextlib import ExitStack

    nc = tc.nc
    f32 = mybir.dt.float32
    bt = mybir.dt.bfloat16

    with ExitStack() as ctx:
        const = ctx.enter_context(tc.tile_pool(name="const", bufs=1))
        mask_sb = const.tile([P, 896], bt, name="mask_sb")
        ones_sb = const.tile([P, P], bt, name="ones_sb")
        wo_sb = const.tile([P, NEC, 256], bt, name="wo_sb")
        nc.vector.memset(ones_sb[:], 1.0)

        dram = ctx.enter_context(tc.tile_pool(name="dram", bufs=1, space="DRAM"))
        ag_in = [dram.tile([2 * P, 512], bt, name=f"ag_in{j}", tag=f"ag_in{j}")
                 for j in range(NQ - 1)]
        ag_out = [dram.tile([E, 512], bt, name=f"ag_out{j}", tag=f"ag_out{j}",
                            addr_space="Shared") for j in range(NQ - 1)]
        # last chunk: per-head collectives so comm starts as soon as head 0
        # finishes and the tail only waits on head 1's small AG
        ag_in3 = [dram.tile([P, 512], bt, name=f"ag_in3{h}", tag=f"ag_in3{h}")
                  for h in range(2)]
        ag_out3 = [dram.tile([8 * P, 512], bt, name=f"ag_out3{h}",
                             tag=f"ag_out3{h}", addr_space="Shared")
                   for h in range(2)]

        # --- phase 1+2 interleaved: per-chunk qkv -> attention -> AG --------
        with tc.tile_pool(name="ph1c", bufs=1) as ph1c_pool, \
             tc.tile_pool(name="chunk", bufs=1) as ck_pool, \
             tc.tile_pool(name="hid", bufs=2 * NEC) as hid_pool, \
             tc.tile_pool(name="wts", bufs=NEC) as wts_pool, \
             tc.tile_pool(name="g01c", bufs=1) as g01_pool, \
             tc.tile_pool(name="rtmp", bufs=1) as rtmp_pool, \
             tc.tile_pool(name="pt", bufs=5) as pt_pool, \
             tc.tile_pool(name="pts", bufs=2) as pts_pool, \
             tc.tile_pool(name="oout", bufs=2) as oo_pool, \
             tc.tile_pool(name="otin", bufs=2) as oi_pool, \
             tc.tile_pool(name="otin3", bufs=1) as oi3_pool, \
             tc.tile_pool(name="oprj", bufs=2) as op_pool, \
             tc.tile_pool(name="gps", bufs=2, space="PSUM") as gps_pool, \
             tc.tile_pool(name="scps", bufs=2, space="PSUM") as sc_pool, \
             tc.tile_pool(name="otps", bufs=2, space="PSUM") as ot_pool, \
             tc.tile_pool(name="dnps", bufs=2, space="PSUM") as dn_pool:

            pp_pool = gps_pool  # oproj PSUM reuses the (idle-by-then) qkv banks

            cos_sb = ph1c_pool.tile([P, S], bt, name="cos_sb", tag="cos_sb")
            sin_sb = ph1c_pool.tile([P, S], bt, name="sin_sb", tag="sin_sb")


            # per-chunk activations: q/k transposed [d, 512]; v [k-in-tile, 4*128]
            # q is only live from its qkv until its attention -> cycle 2 bufs
            qc = [[ck_pool.tile([P, 512], bt, name=f"qc{h}_{j}", tag=f"qc{h}",
                                bufs=2)
                   for j in range(NQ)] for h in range(2)]
            kc = [[ck_pool.tile([P, 512], bt, name=f"kc{h}_{j}", tag=f"kc{h}_{j}")
                   for j in range(NQ)] for h in range(2)]
            vc = [[ck_pool.tile([P, 512], bt, name=f"vc{h}_{j}", tag=f"vc{h}_{j}")
                   for j in range(NQ)] for h in range(2)]

            # ---- input loads, priority-ordered per DMA ring ----------------
            # critical first: wqk (gpsimd ring) + hidden half-0 (sync/scalar/
            # vector rings round-robin), then wv, cos/sin/masks, hidden
            # half-1, and wo last (only needed by the output projection).
            wqk_sb = []
            wv_sb = []
            hid_sb = [[None, None] for _ in range(NEC)]
            for e in range(NEC):
                wq_tile = wts_pool.tile([P, 512], bt, name=f"wqk_{e}", tag="wqk",
                                        bufs=NEC)
                nc.gpsimd.dma_start(wq_tile[:], wqkT[e * P:(e + 1) * P, :])
                wqk_sb.append(wq_tile)
                ht = hid_pool.tile([P, 1024], bt, name=f"hid_{e}_0",
                                   tag="hid", bufs=2 * NEC)
                heng = nc.sync if e % 2 == 0 else nc.scalar
                heng.dma_start(ht[:], hiddenT[e * P:(e + 1) * P, 0:1024])
                hid_sb[e][0] = ht
            nc.gpsimd.dma_start(cos_sb[:], cosT)
            nc.gpsimd.dma_start(sin_sb[:], sinT)
            for e in range(NEC):
                wv_tile = wts_pool.tile([P, 256], bt, name=f"wv_{e}", tag="wv",
                                        bufs=NEC)
                nc.gpsimd.dma_start(wv_tile[:], wvT[e * P:(e + 1) * P, :])
                wv_sb.append(wv_tile)
            nc.gpsimd.dma_start(mask_sb[:], masksT)
            for e in range(NEC):
                ht = hid_pool.tile([P, 1024], bt, name=f"hid_{e}_1",
                                   tag="hid", bufs=2 * NEC)
                heng = nc.sync if e % 2 == 0 else nc.scalar
                heng.dma_start(ht[:], hiddenT[e * P:(e + 1) * P, 1024:2048])
                hid_sb[e][1] = ht
            nc.gpsimd.dma_start(wo_sb[:], woutT.rearrange("p (o e) -> p o e",
                                                          o=NEC))

            oti_sb = []
            oti3_sb = []

            def hidsl(e, lo, width):  # [lo, lo+width) columns of chunk e
                half = hid_sb[e][lo // 1024]
                off = lo % 1024
                return half[:, off:off + width]

            def do_oproj(jj):
                qs = slice(512 * jj, 512 * (jj + 1))
                pps = [pp_pool.tile([P, 512], f32, name=f"pps{jj}_{b}",
                                    tag="gps") for b in range(2)]
                for fc in range(NEC):
                    for b in range(2):
                        nc.tensor.matmul(
                            pps[b][:], wo_sb[:, fc, b * P:(b + 1) * P],
                            oti_sb[jj][:, fc, :], start=(fc == 0),
                            stop=(fc == NEC - 1))
                for b in range(2):
                    ob = op_pool.tile([P, 512], bt, name=f"ob{jj}_{b}",
                                      tag="ob")
                    nc.scalar.copy(ob[:], pps[b][:])
                    nc.sync.dma_start(outT[b * P:(b + 1) * P, qs], ob[:])

            def do_oproj3():
                # chunk 3: even fc rows come from the head-0 AG, odd fc rows
                # from the head-1 AG; accumulate both phases into one PSUM
                # chain so the even half can run while head-1's AG is in
                # flight.
                qs = slice(1536, 2048)
                pps = [pp_pool.tile([P, 512], f32, name=f"pps3_{b}", tag="gps")
                       for b in range(2)]
                for phase in range(2):
                    for k in range(8):
                        fc = 2 * k + phase
                        for b in range(2):
                            nc.tensor.matmul(
                                pps[b][:], wo_sb[:, fc, b * P:(b + 1) * P],
                                oti3_sb[phase][:, k, :],
                                start=(phase == 0 and k == 0),
                                stop=(phase == 1 and k == 7))
                for b in range(2):
                    ob = op_pool.tile([P, 512], bt, name=f"ob3_{b}", tag="ob")
                    nc.scalar.copy(ob[:], pps[b][:])
                    nc.sync.dma_start(outT[b * P:(b + 1) * P, qs], ob[:])

            def do_qkv(j):
                qs = slice(512 * j, 512 * (j + 1))
                # ---- qkv chunk j: rot groups in 2 waves of 2 psum banks ----
                g01 = []
                for g in (0, 1):
                    gp = gps_pool.tile([P, 512], f32, name=f"gps{j}_{g}",
                                       tag="gps")
                    for e in range(NEC):
                        nc.tensor.matmul(
                            gp[:], wqk_sb[e][:, g * P:(g + 1) * P],
                            hidsl(e, 512 * j, 512), start=(e == 0),
                            stop=(e == NEC - 1))
                    gc = g01_pool.tile([P, 512], bt, name=f"g01_{j}_{g}",
                                       tag=f"g01_{g}")
                    nc.scalar.copy(gc[:], gp[:])
                    g01.append(gc)
                for g in (2, 3):
                    gp = gps_pool.tile([P, 512], f32, name=f"gps{j}_{g}",
                                       tag="gps")
                    for e in range(NEC):
                        nc.tensor.matmul(
                            gp[:], wqk_sb[e][:, g * P:(g + 1) * P],
                            hidsl(e, 512 * j, 512), start=(e == 0),
                            stop=(e == NEC - 1))
                    dst = qc if g == 2 else kc
                    nc.vector.tensor_copy(dst[0][j][64:128, :], gp[0:64, :])
                    nc.vector.tensor_copy(dst[1][j][64:128, :], gp[64:128, :])
                # v chunk j
                for st in range(4):
                    vp = gps_pool.tile([P, 256], f32, name=f"vps{j}_{st}",
                                       tag="gps")
                    for e in range(NEC):
                        nc.tensor.matmul(
                            vp[:], hidsl(e, 512 * j + st * P, P),
                            wv_sb[e][:], start=(e == 0), stop=(e == NEC - 1))
                    nc.vector.tensor_copy(vc[0][j][:, st * P:(st + 1) * P],
                                          vp[:, 0:P])
                    nc.vector.tensor_copy(vc[1][j][:, st * P:(st + 1) * P],
                                          vp[:, P:2 * P])
                # rope chunk j
                t0 = rtmp_pool.tile([P, 512], bt, name=f"t0_{j}", tag="t0")
                t1 = rtmp_pool.tile([P, 512], bt, name=f"t1_{j}", tag="t1")
                ta = rtmp_pool.tile([P, 512], bt, name=f"ta_{j}", tag="ta")
                tb = rtmp_pool.tile([P, 512], bt, name=f"tb_{j}", tag="tb")
                nc.vector.tensor_mul(t0[:], g01[0][:], cos_sb[:, qs])
                nc.vector.tensor_mul(t1[:], g01[1][:], sin_sb[:, qs])
                nc.vector.tensor_sub(ta[:], t0[:], t1[:])      # tops
                nc.vector.tensor_mul(t0[:], g01[1][:], cos_sb[:, qs])
                nc.vector.tensor_mul(t1[:], g01[0][:], sin_sb[:, qs])
                nc.vector.tensor_add(tb[:], t0[:], t1[:])      # bottoms
                for pc, dst in enumerate((qc[0], qc[1], kc[0], kc[1])):
                    ps = slice(32 * pc, 32 * (pc + 1))
                    nc.vector.tensor_copy(dst[j][0:32, :], ta[ps, :])
                    nc.vector.tensor_copy(dst[j][32:64, :], tb[ps, :])

            def do_attn(j, hi):
                nk = 4 * j + 4
                otp = ot_pool.tile([P, 512], f32, name=f"otp{hi}_{j}",
                                   tag="otp")
                dnp = dn_pool.tile([P, 512], f32, name=f"dnp{hi}_{j}",
                                   tag="dnp")
                prev_pt, prev_lo = None, 0
                for i in range(nk):
                    # on diagonal k-tiles only q-columns >= 128*mm are alive:
                    # every op is trimmed to that range (cols below are fully
                    # masked and the first, full-width tile of each chain
                    # already initialized them)
                    mm = i - 4 * j
                    lo = 128 * mm if mm > 0 else 0
                    scp = sc_pool.tile([P, 512], f32, name=f"scp{hi}{j}{i}",
                                       tag="scp")
                    nc.tensor.matmul(
                        scp[:, lo:], kc[hi][i // 4][:, (i % 4) * P:
                                                    (i % 4 + 1) * P],
                        qc[hi][j][:, lo:], start=True, stop=True)
                    pt = pt_pool.tile([P, 512], bt, name=f"pt{hi}{j}{i}",
                                      tag="pt")
                    nc.scalar.activation(
                        pt[:, lo:], scp[:, lo:],
                        mybir.ActivationFunctionType.Exp, scale=SCALE)
                    if mm >= 0:
                        nc.vector.tensor_mul(
                            pt[:, lo:], pt[:, lo:], mask_sb[:, 384:896 - lo])
                    nc.tensor.matmul(
                        otp[:, lo:], vc[hi][i // 4][:, (i % 4) * P:
                                                    (i % 4 + 1) * P],
                        pt[:, lo:],
                        start=(i == 0), stop=(i == nk - 1),
                        skip_group_check=True)
                    # denominator: pre-add pt pairs on the DVE (bf16, cheap)
                    # so only nk/2 ones-matmuls hit the PE
                    if i % 2 == 1:
                        ps2 = pts_pool.tile([P, 512], bt,
                                            name=f"ps2{hi}{j}{i}", tag="ps2")
                        if prev_lo < lo:
                            nc.vector.tensor_copy(ps2[:, prev_lo:lo],
                                                  prev_pt[:, prev_lo:lo])
                        nc.vector.tensor_add(ps2[:, lo:], prev_pt[:, lo:],
                                             pt[:, lo:])
                        nc.tensor.matmul(
                            dnp[:, prev_lo:], ones_sb[:], ps2[:, prev_lo:],
                            start=(i == 1), stop=(i == nk - 1),
                            skip_group_check=True)
                    prev_pt, prev_lo = pt, lo
                den_sb = oo_pool.tile([P, 512], f32, name=f"den{hi}_{j}",
                                      tag="den")
                nc.vector.reciprocal_approx_fast(den_sb[:], dnp[:])
                otn = oo_pool.tile([P, 512], bt, name=f"otn{hi}_{j}",
                                   tag="otn")
                nc.vector.tensor_mul(otn[:], otp[:], den_sb[:])
                if j < NQ - 1:
                    nc.sync.dma_start(ag_in[j][hi * P:(hi + 1) * P, :],
                                      otn[:])
                else:
                    nc.sync.dma_start(ag_in3[hi][:], otn[:])

            def prefetch_oti(j, pieces=1):
                # stage gathered O^T; pieces let the consumer start on piece
                # 0 while later pieces stream
                oti = oi_pool.tile([P, NEC, 512], bt, name=f"oti{j}",
                                   tag="oti")
                w = NEC // pieces
                for k in range(pieces):
                    nc.gpsimd.dma_start(
                        oti[:, w * k:w * (k + 1), :],
                        ag_out[j][P * w * k:P * w * (k + 1), :].rearrange(
                            "(o p) s -> p o s", p=P))
                oti_sb.append(oti)

            def prefetch_oti3(hi, pieces=1):
                oti3 = oi3_pool.tile([P, 8, 512], bt, name=f"oti3{hi}",
                                     tag=f"oti3{hi}")
                w = 8 // pieces
                for k in range(pieces):
                    nc.gpsimd.dma_start(
                        oti3[:, w * k:w * (k + 1), :],
                        ag_out3[hi][P * w * k:P * w * (k + 1), :].rearrange(
                            "(o p) s -> p o s", p=P))
                oti3_sb.append(oti3)

            def fire_ag(j, prefetch=True):
                nc.gpsimd.collective_compute(
                    "AllGather",
                    mybir.AluOpType.bypass,
                    replica_groups=[list(range(NCORES))],
                    ins=[ag_in[j].opt()],
                    outs=[ag_out[j].opt()],
                )
                if prefetch:
                    prefetch_oti(j)

            def fire_ag3(hi, prefetch=True):
                nc.gpsimd.collective_compute(
                    "AllGather",
                    mybir.AluOpType.bypass,
                    replica_groups=[list(range(NCORES))],
                    ins=[ag_in3[hi].opt()],
                    outs=[ag_out3[hi].opt()],
                )
                if prefetch:
                    prefetch_oti3(hi)

            # schedule: chunks 0/1 pipelined; chunk 3 head 0's AG fires while
            # chunk 2's attention runs, chunk 2's AG while chunk 3 head 1
            # runs, so the CC engine stays busy and the final AG is the small
            # per-head one; all output projections run at the end, filling
            # the last AG's latency with PE work.
            do_qkv(0)
            do_attn(0, 0)
            do_attn(0, 1)
            fire_ag(0)
            do_qkv(1)
            do_attn(1, 0)
            do_attn(1, 1)
            fire_ag(1)
            do_qkv(2)
            do_qkv(3)
            do_attn(3, 0)
            fire_ag3(0)
            do_attn(2, 0)
            do_attn(2, 1)
            fire_ag(2, prefetch=False)
            do_attn(3, 1)
            fire_ag3(1, prefetch=False)
            # prefetches after both AG issues: a DIRECT2D blocked on its
            # AG-done semaphore must never delay an AG issue behind it
            prefetch_oti(2, pieces=4)
            prefetch_oti3(1, pieces=2)
            do_oproj(0)
            do_oproj(1)
            do_oproj(2)
            do_oproj3()


def _build_program():
    import concourse.bass as bass  # noqa: F401
    import concourse.mybir as mybir
    import concourse.tile as tile
    from concourse import bacc

    nc = bacc.Bacc("TRN2", target_bir_lowering=False, debug=False,
                   enable_asserts=False, num_devices=NCORES)
    f32 = mybir.dt.float32
    bt = mybir.dt.bfloat16
    hiddenT = nc.dram_tensor("hiddenT", [E, S], bt, kind="ExternalInput").ap()
    wqkT = nc.dram_tensor("wqkT", [E, 512], bt, kind="ExternalInput").ap()
    wvT = nc.dram_tensor("wvT", [E, 256], bt, kind="ExternalInput").ap()
    woutT = nc.dram_tensor("woutT", [P, NEC * 256], bt, kind="ExternalInput").ap()
    cosT = nc.dram_tensor("cosT", [P, S], bt, kind="ExternalInput").ap()
    sinT = nc.dram_tensor("sinT", [P, S], bt, kind="ExternalInput").ap()
    masks = nc.dram_tensor("masks", [P, 896], bt, kind="ExternalInput").ap()
    outT = nc.dram_tensor("outT", [2 * P, S], bt, kind="ExternalOutput").ap()

    with tile.TileContext(nc) as tc:
        _kernel_body(tc, outT, hiddenT, wqkT, wvT, woutT, cosT, sinT, masks)
    nc.compile()
    return nc


def get_program():
    if "nc" not in _CACHE:
        _CACHE["nc"] = _build_program()
    return _CACHE["nc"]


def _install_ntff_shim():
    """Provide antenv.axon_hooks (missing in this image) so trace=True works."""
    import sys
    import types
    try:
        import antenv.axon_hooks  # noqa: F401
        return
    except ImportError:
        pass
    import antenv
    mod = types.ModuleType("antenv.axon_hooks")
    mod._hook = None

    def set_axon_ntff_profile_hook(h):
        mod._hook = h

    def get_axon_ntff_profile_hook():
        return mod._hook

    mod.set_axon_ntff_profile_hook = set_axon_ntff_profile_hook
    mod.get_axon_ntff_profile_hook = get_axon_ntff_profile_hook
    sys.modules["antenv.axon_hooks"] = mod
    antenv.axon_hooks = mod
    try:
        from trn_agent_boot.trn_boot import _ntff_profile_via_ctypes
        hook = _ntff_profile_via_ctypes("/opt/axon/libaxon_pjrt.so")
        if hook is not None:
            mod._hook = hook
    except Exception:
        pass


def run(inputs, trace=False):
    """Run on the 8 NeuronCores; returns (out [1,S,E], BassKernelResults)."""
    from concourse import bass_utils

    if trace:
        _install_ntff_shim()
    nc = get_program()
    in_maps = _build_in_maps(inputs["hidden_states"], inputs["w_qkv"],
                             inputs["w_out"])
    res = bass_utils.run_bass_kernel_spmd(
        nc, in_maps, core_ids=list(range(NCORES)), trace=trace)
    outT = np.concatenate(
        [np.asarray(res.results[c]["outT"], dtype=np.float32)
         for c in range(NCORES)], axis=0)  # [E, S]
    out = np.ascontiguousarray(outT.T).reshape(1, S, E).astype(np.float32)
    return out, res


def kernel(hidden_states, w_qkv, w_out):
    out, _ = run({"hidden_states": hidden_states, "w_qkv": w_qkv,
                  "w_out": w_out})
    return out
